# revision 1
# baseline (speedup 1.0000x reference)
import sys

for _p in ("/opt/trn_rl_repo",):
    if _p not in sys.path:
        sys.path.append(_p)

"""AttnBlock (GroupNorm + single-head self-attention + residual) Bass/Tile
kernel for one NeuronCore (one batch sample), channel-major layout.

Per-core problem:  x [C=512, HW] f32
  hn = groupnorm(x, 32 groups, eps=1e-5) * gn_w + gn_b
  q/k/v = 1x1 conv (C x C) on tokens;  scores = (q k^T) / sqrt(C)
  attn = softmax(scores);  o = attn @ v;  out = x + (o @ wo^T + bo)

Layout strategy (matmuls in float32r, TF32-like, ~1 cycle/row):
  - hn, Qt, Kt channel-major [c, hw];  V token-major [hw, c]
  - scores computed transposed St[j, q] = sum_c Kt[c,j] Qt[c,q]
  - exp via ACT, no max subtraction (scores ~N(0,1) by construction)
  - softmax denominator: elementwise accumulate exp tiles on DVE, then a
    ones-vector matmul for the partition sum; normalization applied to
    O^T after the PV accumulation (rank-1 ones matmul broadcasts 1/d)
  - PV: O^T[c, q] += V[j, :]^T P^T[j, q] accumulated in PSUM over j
  - K/V split in two j-halves to fit SBUF; half 2 spilled to DRAM in
    phase A and reloaded for pass B2; partial O/denoms of B1 spilled.
"""

from contextlib import ExitStack

import concourse.bass as bass
import concourse.tile as tile
from concourse import mybir
from concourse.masks import make_identity

F32 = mybir.dt.float32
F32R = mybir.dt.float32r
AX = mybir.AxisListType
OP = mybir.AluOpType
ACTF = mybir.ActivationFunctionType

C = 512
NCH = 4  # channel chunks of 128
GPC = 8  # groups per 128-channel chunk (16 channels per group)
EPS = 1e-5


def build(nc: bass.Bass, HW: int = 4096):
    SCALE_Q = float(C) ** (-0.5)
    NJB = HW // 512      # j blocks (phase A streaming)
    NQB = HW // 512      # q blocks (phase B)
    JBR = NJB // 2       # j blocks in the resident (first) half
    HW2 = HW // 2
    NJT2 = HW2 // 128    # j tiles per half
    KROWS = NCH * JBR    # 512-wide rows of Kt half in the packed kv tile
    GN_N = 16 * HW       # elements per group

    x = nc.dram_tensor("x", [C, HW], F32, kind="ExternalInput")
    gn_w = nc.dram_tensor("gn_w", [C], F32, kind="ExternalInput")
    gn_b = nc.dram_tensor("gn_b", [C], F32, kind="ExternalInput")
    wq = nc.dram_tensor("wq", [C, C], F32, kind="ExternalInput")
    bq = nc.dram_tensor("bq", [C], F32, kind="ExternalInput")
    wk = nc.dram_tensor("wk", [C, C], F32, kind="ExternalInput")
    bk = nc.dram_tensor("bk", [C], F32, kind="ExternalInput")
    wv = nc.dram_tensor("wv", [C, C], F32, kind="ExternalInput")
    bv = nc.dram_tensor("bv", [C], F32, kind="ExternalInput")
    wo = nc.dram_tensor("wo", [C, C], F32, kind="ExternalInput")
    bo = nc.dram_tensor("bo", [C], F32, kind="ExternalInput")
    out = nc.dram_tensor("out", [C, HW], F32, kind="ExternalOutput")

    # internal DRAM spill buffers
    qt_dram = nc.dram_tensor("qt_spill", [128, NCH, HW], F32R)
    kt2_dram = nc.dram_tensor("kt2_spill", [128, NCH, HW2], F32R)
    v2_dram = nc.dram_tensor("v2_spill", [128, NJT2, 512], F32R)
    o1_dram = nc.dram_tensor("o1_spill", [128, NCH, HW], F32)
    d1_dram = nc.dram_tensor("d1_spill", [HW // 512, 512], F32)

    x_r = x.rearrange("(c p) q -> p c q", p=128)
    out_r = out.rearrange("(c p) q -> p c q", p=128)

    def kv_views(kv):
        kt = kv[:, 0:KROWS, :].rearrange("p (c j) w -> p c (j w)", c=NCH)
        v = kv[:, KROWS:, :]
        return kt, v

    with tile.TileContext(nc) as tc, ExitStack() as ctx:
        pconst = ctx.enter_context(tc.tile_pool(name="const", bufs=1))
        ppersist = ctx.enter_context(tc.tile_pool(name="persist", bufs=1))
        pstream = ctx.enter_context(tc.tile_pool(name="stream", bufs=2))
        pkv = ctx.enter_context(tc.tile_pool(name="kv", bufs=1))

        # ---- constants ----
        identity = pconst.tile([128, 128], F32, tag="ident")
        make_identity(nc, identity[:])
        ones128_f = pconst.tile([128, 1], F32, tag="ones128_f")
        nc.gpsimd.memset(ones128_f[:], 1.0)
        ones128 = pconst.tile([128, 1], F32R, tag="ones128")
        nc.vector.tensor_copy(ones128[:], ones128_f[:])
        ones1_f = pconst.tile([1, 128], F32, tag="ones1_f")
        nc.gpsimd.memset(ones1_f[:], 1.0)
        ones1 = pconst.tile([1, 128], F32R, tag="ones1")
        nc.vector.tensor_copy(ones1[:], ones1_f[:])
        # group indicator matrices: ind8[c, g] = e8[g, c] = (c // 16 == g)
        # built as a range test 0 <= c - 16 g <= 15 via two affine selects
        ind8_f = pconst.tile([128, GPC], F32, tag="ind8_f")
        nc.gpsimd.memset(ind8_f[:], 1.0)
        nc.gpsimd.affine_select(
            out=ind8_f[:], in_=ind8_f[:], compare_op=OP.is_ge, fill=0.0,
            base=0, channel_multiplier=1, pattern=[[-16, GPC]],
        )
        nc.gpsimd.affine_select(
            out=ind8_f[:], in_=ind8_f[:], compare_op=OP.is_ge, fill=0.0,
            base=15, channel_multiplier=-1, pattern=[[16, GPC]],
        )
        ind8 = pconst.tile([128, GPC], F32R, tag="ind8")
        nc.vector.tensor_copy(ind8[:], ind8_f[:])
        e8_f = pconst.tile([GPC, 128], F32, tag="e8_f")
        nc.gpsimd.memset(e8_f[:], 1.0)
        nc.gpsimd.affine_select(
            out=e8_f[:], in_=e8_f[:], compare_op=OP.is_ge, fill=0.0,
            base=0, channel_multiplier=-16, pattern=[[1, 128]],
        )
        nc.gpsimd.affine_select(
            out=e8_f[:], in_=e8_f[:], compare_op=OP.is_ge, fill=0.0,
            base=15, channel_multiplier=16, pattern=[[-1, 128]],
        )
        e8 = pconst.tile([GPC, 128], F32R, tag="e8")
        nc.vector.tensor_copy(e8[:], e8_f[:])

        gnw4 = pconst.tile([128, NCH], F32, tag="gnw4")
        gnb4 = pconst.tile([128, NCH], F32, tag="gnb4")
        bq4 = pconst.tile([128, NCH], F32, tag="bq4")
        bqs4 = pconst.tile([128, NCH], F32, tag="bqs4")
        bk4 = pconst.tile([128, NCH], F32, tag="bk4")
        bo4 = pconst.tile([128, NCH], F32, tag="bo4")
        for t, src in ((gnw4, gn_w), (gnb4, gn_b), (bq4, bq), (bk4, bk), (bo4, bo)):
            nc.sync.dma_start(out=t[:], in_=src.rearrange("(c p) -> p c", p=128))
        nc.vector.tensor_scalar_mul(bqs4[:], bq4[:], SCALE_Q)
        bv_row = pconst.tile([1, C], F32, tag="bv_row")
        nc.sync.dma_start(out=bv_row[:], in_=bv.rearrange("(a i) -> a i", a=1))
        bv_row_r = pconst.tile([1, C], F32R, tag="bv_row_r")
        nc.vector.tensor_copy(bv_row_r[:], bv_row[:])
        bv_bcast = pconst.tile([128, C], F32, tag="bv_bcast")

        eps_t = pconst.tile([GPC, 1], F32, tag="eps_t")
        nc.gpsimd.memset(eps_t[:], EPS)
        sum_cols = pconst.tile([128, NCH, NJB], F32, tag="sum_cols")
        sq_cols = pconst.tile([128, NCH, NJB], F32, tag="sq_cols")
        ch_stats_r = pconst.tile([128, NCH, 2], F32R, tag="ch_stats_r")
        scale4 = pconst.tile([128, NCH], F32, tag="scale4")
        shift4 = pconst.tile([128, NCH], F32, tag="shift4")

        # ---- persistent tensors ----
        woT = ppersist.tile([128, NCH, C], F32R, tag="woT")
        kv1 = pkv.tile([128, KROWS + NJT2, 512], F32R, tag="kv")
        kv1_kt, kv1_v = kv_views(kv1)

        # ---- phase A ----
        with tc.tile_pool(name="wqkv", bufs=1) as pwqkv:
            wqT = pwqkv.tile([128, NCH, C], F32R, tag="wqT")
            wkT = pwqkv.tile([128, NCH, C], F32R, tag="wkT")
            wvT = pwqkv.tile([128, NCH, C], F32R, tag="wvT")

            with tc.tile_pool(name="psA", bufs=1, space="PSUM") as psA:
                with tc.tile_pool(name="scrA", bufs=2) as pscr:
                    # ---- pass 1: GN statistics ----
                    for jb in range(NJB):
                        x_in = pstream.tile([128, NCH, 512], F32, tag="xin")
                        nc.sync.dma_start(
                            out=x_in[:], in_=x_r[:, :, 512 * jb : 512 * (jb + 1)]
                        )
                        for ci in range(NCH):
                            nc.vector.reduce_sum(
                                sum_cols[:, ci, jb : jb + 1], x_in[:, ci, :], axis=AX.X
                            )
                            xsq = pscr.tile([128, 512], F32, tag="xsq")
                            nc.scalar.activation(
                                xsq[:],
                                x_in[:, ci, :],
                                ACTF.Square,
                                accum_out=sq_cols[:, ci, jb : jb + 1],
                            )
                    # weight transposes: wT[:, ci, co*128:..] = W[co blk, ci blk].T
                    with tc.tile_pool(name="raw", bufs=2) as praw:
                        for w_ext, wT in ((wq, wqT), (wk, wkT), (wv, wvT), (wo, woT)):
                            raw = praw.tile([128, NCH, C], F32, tag="raw")
                            nc.sync.dma_start(
                                out=raw[:], in_=w_ext.rearrange("(c p) i -> p c i", p=128)
                            )
                            for co in range(NCH):
                                for ci in range(NCH):
                                    ps = psA.tile([128, 128], F32, tag="m", bufs=4)
                                    nc.tensor.transpose(
                                        ps[:],
                                        raw[:, co, 128 * ci : 128 * (ci + 1)],
                                        identity[:],
                                    )
                                    nc.scalar.activation(
                                        wT[:, ci, 128 * co : 128 * (co + 1)],
                                        ps[:],
                                        ACTF.Identity,
                                    )
                        # bv broadcast tile (rank-1 matmul)
                        psbv = psA.tile([128, C], F32, tag="m", bufs=4)
                        nc.tensor.matmul(
                            psbv[:], ones1[:], bv_row_r[:], start=True, stop=True
                        )
                        nc.scalar.activation(bv_bcast[:], psbv[:], ACTF.Identity)
                    # combine stats -> per-channel scale/shift
                    for ci in range(NCH):
                        with nc.allow_low_precision(
                            reason="f32r rounding of GN sums is ~2^-11 relative"
                        ):
                            nc.vector.reduce_sum(
                                ch_stats_r[:, ci, 0:1], sum_cols[:, ci, :], axis=AX.X
                            )
                            nc.vector.reduce_sum(
                                ch_stats_r[:, ci, 1:2], sq_cols[:, ci, :], axis=AX.X
                            )
                        psg = psA.tile([GPC, 2], F32, tag="t", bufs=2)
                        nc.tensor.matmul(
                            psg[:], ind8[:], ch_stats_r[:, ci, :], start=True, stop=True
                        )
                        mean = pscr.tile([GPC, 1], F32, tag="st_mean")
                        ex2 = pscr.tile([GPC, 1], F32, tag="st_ex2")
                        nc.vector.tensor_scalar_mul(mean[:], psg[:, 0:1], 1.0 / GN_N)
                        nc.vector.tensor_scalar_mul(ex2[:], psg[:, 1:2], 1.0 / GN_N)
                        var = pscr.tile([GPC, 1], F32, tag="st_var")
                        nc.vector.tensor_mul(var[:], mean[:], mean[:])
                        nc.vector.tensor_sub(var[:], ex2[:], var[:])
                        std = pscr.tile([GPC, 1], F32, tag="st_std")
                        nc.scalar.activation(std[:], var[:], ACTF.Sqrt, bias=eps_t[:])
                        rstd = pscr.tile([GPC, 1], F32, tag="st_rstd")
                        nc.vector.reciprocal(rstd[:], std[:])
                        st2 = pscr.tile([GPC, 2], F32R, tag="st2")
                        nc.vector.tensor_copy(st2[:, 0:1], rstd[:])
                        nc.vector.tensor_copy(st2[:, 1:2], mean[:])
                        pse = psA.tile([128, 2], F32, tag="t", bufs=2)
                        nc.tensor.matmul(pse[:], e8[:], st2[:], start=True, stop=True)
                        # scale = rstd * gamma ; shift = beta - mean * scale
                        nc.vector.tensor_mul(
                            scale4[:, ci : ci + 1], pse[:, 0:1], gnw4[:, ci : ci + 1]
                        )
                        tmp = pscr.tile([128, 1], F32, tag="st_tmp")
                        nc.vector.tensor_mul(
                            tmp[:], pse[:, 1:2], scale4[:, ci : ci + 1]
                        )
                        nc.vector.tensor_sub(
                            shift4[:, ci : ci + 1], gnb4[:, ci : ci + 1], tmp[:]
                        )

                    # ---- pass 2: GN apply + Q/K/V projections ----
                    for jb in range(NJB):
                        x_in = pstream.tile([128, NCH, 512], F32, tag="xin")
                        nc.sync.dma_start(
                            out=x_in[:], in_=x_r[:, :, 512 * jb : 512 * (jb + 1)]
                        )
                        hn = pscr.tile([128, NCH, 512], F32R, tag="hn")
                        for ci in range(NCH):
                            nc.scalar.activation(
                                hn[:, ci, :],
                                x_in[:, ci, :],
                                ACTF.Identity,
                                scale=scale4[:, ci : ci + 1],
                                bias=shift4[:, ci : ci + 1],
                            )
                        # Q -> spill to DRAM (scaled by 1/sqrt(C))
                        qstag = pscr.tile([128, NCH, 512], F32R, tag="qstag")
                        for co in range(NCH):
                            psq = psA.tile([128, 512], F32, tag="m", bufs=4)
                            for ci in range(NCH):
                                nc.tensor.matmul(
                                    psq[:],
                                    wqT[:, ci, 128 * co : 128 * (co + 1)],
                                    hn[:, ci, :],
                                    start=(ci == 0),
                                    stop=(ci == NCH - 1),
                                )
                            nc.scalar.activation(
                                qstag[:, co, :],
                                psq[:],
                                ACTF.Identity,
                                scale=SCALE_Q,
                                bias=bqs4[:, co : co + 1],
                            )
                        nc.sync.dma_start(
                            out=qt_dram[:, :, 512 * jb : 512 * (jb + 1)], in_=qstag[:]
                        )
                        # K -> resident (first half) or staged+spilled
                        kstag = None
                        if jb >= JBR:
                            kstag = pscr.tile([128, NCH, 512], F32R, tag="stag")
                        for co in range(NCH):
                            psk = psA.tile([128, 512], F32, tag="m", bufs=4)
                            for ci in range(NCH):
                                nc.tensor.matmul(
                                    psk[:],
                                    wkT[:, ci, 128 * co : 128 * (co + 1)],
                                    hn[:, ci, :],
                                    start=(ci == 0),
                                    stop=(ci == NCH - 1),
                                )
                            kdst = (
                                kv1_kt[:, co, 512 * jb : 512 * (jb + 1)]
                                if jb < JBR
                                else kstag[:, co, :]
                            )
                            nc.scalar.activation(
                                kdst, psk[:], ACTF.Identity, bias=bk4[:, co : co + 1]
                            )
                        if jb >= JBR:
                            nc.sync.dma_start(
                                out=kt2_dram[
                                    :, :, 512 * (jb - JBR) : 512 * (jb - JBR + 1)
                                ],
                                in_=kstag[:],
                            )
                        # V[j, c] per j-subtile -> resident or staged+spilled
                        vstag = None
                        if jb >= JBR:
                            vstag = pscr.tile([128, NCH, 512], F32R, tag="stag")
                        for jtl in range(4):
                            psv = psA.tile([128, 512], F32, tag="m", bufs=4)
                            for ci in range(NCH):
                                nc.tensor.matmul(
                                    psv[:],
                                    hn[:, ci, 128 * jtl : 128 * (jtl + 1)],
                                    wvT[:, ci, :],
                                    start=(ci == 0),
                                    stop=(ci == NCH - 1),
                                )
                            vdst = (
                                kv1_v[:, 4 * jb + jtl, :]
                                if jb < JBR
                                else vstag[:, jtl, :]
                            )
                            nc.vector.tensor_add(vdst, psv[:], bv_bcast[:])
                        if jb >= JBR:
                            nc.sync.dma_start(
                                out=v2_dram[:, 4 * (jb - JBR) : 4 * (jb - JBR + 1), :],
                                in_=vstag[:],
                            )

        # ---- phase B ----
        with (
            tc.tile_pool(name="poolB", bufs=1) as pB,
            tc.tile_pool(name="psB", bufs=1, space="PSUM") as psB,
        ):
            pending = None

            def emit_epilogue(p):
                # deferred final projection + bias + residual for a prior
                # q-block; spliced into the next q-block's PE stream so it
                # fills the scores->exp->PV latency bubble
                e_qb, e_osb, e_rbc, e_xb = p
                outs = pB.tile([128, NCH, 512], F32, tag="outs", bufs=2)
                for co in range(NCH):
                    psf = psB.tile([128, 512], F32, tag="f", bufs=2)
                    for cc in range(NCH):
                        nc.tensor.matmul(
                            psf[:],
                            woT[:, cc, 128 * co : 128 * (co + 1)],
                            e_osb[:, cc, :],
                            start=(cc == 0),
                            stop=(cc == NCH - 1),
                        )
                    nc.vector.tensor_mul(outs[:, co, :], psf[:], e_rbc[:])
                    nc.vector.tensor_add(
                        outs[:, co, :], outs[:, co, :], e_xb[:, co, :]
                    )
                nc.sync.dma_start(
                    out=out_r[:, :, 512 * e_qb : 512 * (e_qb + 1)], in_=outs[:]
                )

            for half in range(2):
                if half == 0:
                    kt_t, v_t = kv1_kt, kv1_v
                else:
                    kv2 = pkv.tile([128, KROWS + NJT2, 512], F32R, tag="kv")
                    kt_t, v_t = kv_views(kv2)
                    # interleave the reload in j-order chunks so the first
                    # j-tiles of B2 can start before the whole 8MB lands
                    for jbl in range(JBR):
                        nc.sync.dma_start(
                            out=kt_t[:, :, 512 * jbl : 512 * (jbl + 1)],
                            in_=kt2_dram[:, :, 512 * jbl : 512 * (jbl + 1)],
                        )
                        nc.sync.dma_start(
                            out=v_t[:, 4 * jbl : 4 * (jbl + 1), :],
                            in_=v2_dram[:, 4 * jbl : 4 * (jbl + 1), :],
                        )
                for qb in range(NQB):
                    qt_in = pB.tile([128, NCH, 512], F32R, tag="qt_in", bufs=2)
                    nc.sync.dma_start(
                        out=qt_in[:], in_=qt_dram[:, :, 512 * qb : 512 * (qb + 1)]
                    )
                    # den accumulated directly in f32r so the ones-matmul can
                    # consume it without a rounding copy
                    den = pB.tile([128, 512], F32R, tag="den", bufs=2)
                    pso = [
                        psB.tile([128, 512], F32, tag="o", bufs=4, name="pso") for _ in range(NCH)
                    ]
                    for jt in range(NJT2):
                        pss = psB.tile([128, 512], F32, tag="s", bufs=2)
                        for ci in range(NCH):
                            nc.tensor.matmul(
                                pss[:],
                                kt_t[:, ci, 128 * jt : 128 * (jt + 1)],
                                qt_in[:, ci, :],
                                start=(ci == 0),
                                stop=(ci == NCH - 1),
                            )
                        if jt == 0 and pending is not None:
                            emit_epilogue(pending)
                            pending = None
                        pt = pB.tile([128, 512], F32R, tag="pt", bufs=3)
                        nc.scalar.activation(pt[:], pss[:], ACTF.Exp)
                        ptf = pt[:].bitcast(F32)
                        if jt == 0:
                            nc.vector.tensor_copy(den[:], ptf)
                        else:
                            nc.vector.tensor_add(den[:], den[:].bitcast(F32), ptf)
                        for cc in range(NCH):
                            nc.tensor.matmul(
                                pso[cc][:],
                                v_t[:, jt, 128 * cc : 128 * (cc + 1)],
                                pt[:],
                                start=(jt == 0),
                                stop=(jt == NJT2 - 1),
                            )
                    psd = psB.tile([1, 512], F32, tag="f", bufs=2)
                    nc.tensor.matmul(psd[:], ones128[:], den[:], start=True, stop=True)
                    if half == 0:
                        # spill partial denom + partial (unnormalized) O
                        den1 = pB.tile([1, 512], F32, tag="den1", bufs=2)
                        nc.scalar.activation(den1[:], psd[:], ACTF.Identity)
                        nc.sync.dma_start(out=d1_dram[qb : qb + 1, :], in_=den1[:])
                        o1s = pB.tile([128, NCH, 512], F32, tag="osb", bufs=2)
                        for cc in range(NCH):
                            nc.scalar.activation(
                                o1s[:, cc, :], pso[cc][:], ACTF.Identity
                            )
                        nc.sync.dma_start(
                            out=o1_dram[:, :, 512 * qb : 512 * (qb + 1)], in_=o1s[:]
                        )
                    else:
                        # merge the B1 partial FIRST (so the PE-blocking osb
                        # adds run ahead of the slow reciprocal in the DVE
                        # queue), then the denominator chain, which overlaps
                        # the final-projection matmuls
                        o1l = pB.tile([128, NCH, 512], F32, tag="osb", bufs=2)
                        nc.sync.dma_start(
                            out=o1l[:], in_=o1_dram[:, :, 512 * qb : 512 * (qb + 1)]
                        )
                        osb = pB.tile([128, NCH, 512], F32R, tag="osbr", bufs=2)
                        for cc in range(NCH):
                            nc.vector.tensor_add(
                                osb[:, cc, :], pso[cc][:], o1l[:, cc, :]
                            )
                        den1 = pB.tile([1, 512], F32, tag="den1", bufs=2)
                        nc.sync.dma_start(out=den1[:], in_=d1_dram[qb : qb + 1, :])
                        dtot = pB.tile([1, 512], F32R, tag="dtot", bufs=2)
                        nc.vector.tensor_add(dtot[:], psd[:], den1[:])
                        psb = psB.tile([128, 512], F32, tag="f", bufs=2)
                        nc.tensor.matmul(
                            psb[:], ones1[:], dtot[:], start=True, stop=True
                        )
                        rbc = pB.tile([128, 512], F32, tag="rbc", bufs=2)
                        nc.vector.reciprocal(rbc[:], psb[:])
                        # bias+residual prepared off the critical path
                        xres = pstream.tile([128, NCH, 512], F32, tag="xin")
                        nc.sync.dma_start(
                            out=xres[:], in_=x_r[:, :, 512 * qb : 512 * (qb + 1)]
                        )
                        xb = pB.tile([128, NCH, 512], F32, tag="xb", bufs=2)
                        for co in range(NCH):
                            nc.scalar.activation(
                                xb[:, co, :],
                                xres[:, co, :],
                                ACTF.Identity,
                                bias=bo4[:, co : co + 1],
                            )
                        pending = (qb, osb, rbc, xb)
            if pending is not None:
                emit_epilogue(pending)
                pending = None
    return nc


# ---------------------------------------------------------------------------
# Walrus in this container rejects instructions carrying more than ~2
# sync-wait commands ("Too many sync wait commands").  Hoist excess on_wait
# entries onto nofuse NOPs placed immediately before the instruction on the
# same engine (engines issue in-order, so blocking on the NOP first is
# equivalent).
def split_sync_waits(nc, max_waits=1):
    n_split = 0
    for bb in nc.main_func.blocks:
        insts = bb.instructions
        out = []
        for inst in insts:
            si = inst.sync_info
            if si is not None and si.on_wait is not None and len(si.on_wait) > max_waits:
                waits = list(si.on_wait)
                keep = waits[-max_waits:]
                extra = waits[:-max_waits]
                for i in range(0, len(extra), max_waits):
                    chunk = extra[i : i + max_waits]
                    nop = mybir.InstNoOp(
                        name=f"{inst.name}-sw{i}",
                        sync_info=mybir.SyncInfo(on_wait=chunk, on_update=[]),
                        bass_nofuse=True,
                        engine=inst.engine,
                    )
                    out.append(nop)
                    n_split += 1
                inst.sync_info = mybir.SyncInfo(
                    on_wait=keep, on_update=list(si.on_update or [])
                )
            out.append(inst)
        bb.instructions = out
    return n_split


B, H, W = 8, 64, 64
HW = H * W
N_CORES = 8
_CACHE = {}


def _get_nc():
    if "nc" not in _CACHE:
        nc = bass.Bass()
        build(nc, HW=HW)
        split_sync_waits(nc)
        _CACHE["nc"] = nc
    return _CACHE["nc"]


def _in_maps(inputs):
    import numpy as np
    arrs = {k: np.ascontiguousarray(np.asarray(v, dtype=np.float32)) for k, v in inputs.items()}
    x = arrs.pop("x").reshape(B, C, HW)
    return [{"x": x[i], **arrs} for i in range(N_CORES)]


def kernel(**inputs):
    import numpy as np
    from concourse.bass_utils import run_bass_kernel_spmd

    nc = _get_nc()
    res = run_bass_kernel_spmd(nc, _in_maps(inputs), list(range(N_CORES)))
    out = np.stack([res.results[i]["out"] for i in range(N_CORES)])
    return out.reshape(B, C, H, W).astype(np.float32)


def kernel_traced(**inputs):
    """Like kernel() but with NTFF profiling; returns (output, BassKernelResults)."""
    import numpy as np
    from concourse.bass_utils import run_bass_kernel_spmd

    nc = _get_nc()
    res = run_bass_kernel_spmd(
        nc, _in_maps(inputs), list(range(N_CORES)), trace=True
    )
    out = np.stack([res.results[i]["out"] for i in range(N_CORES)])
    return out.reshape(B, C, H, W).astype(np.float32), res



# revision 5
# speedup vs baseline: 1.1401x; 1.1401x over previous
import sys

for _p in ("/opt/trn_rl_repo",):
    if _p not in sys.path:
        sys.path.append(_p)

"""AttnBlock (GroupNorm + single-head self-attention + residual) Bass/Tile
kernel for one NeuronCore (one batch sample), channel-major layout.

Per-core problem:  x [C=512, HW] f32
  hn = groupnorm(x, 32 groups, eps=1e-5) * gn_w + gn_b
  q/k/v = 1x1 conv (C x C) on tokens;  scores = (q k^T) / sqrt(C)
  attn = softmax(scores);  o = attn @ v;  out = x + (o @ wo^T + bo)

Layout strategy (all big matmuls in bf16: same 1 col/cycle PE rate as
f32r but enables fast-weight-load so LDWEIGHTS hides behind streaming;
bf16 also halves SBUF so Qt/Kt/V and x all stay resident -> single-pass
phase B with zero DRAM spills):
  - x kept in SBUF as bf16 after the stats pass (residual + pass-2 reads)
  - hn, Qt, Kt channel-major [c, hw];  V token-major [hw, c]
  - scores computed transposed St[j, q] = sum_c Kt[c,j] Qt[c,q]
  - exp via ACT, no max subtraction (scores ~N(0,1) by construction)
  - softmax denominator: elementwise accumulate exp tiles on DVE, then a
    ones-vector matmul for the partition sum; normalization applied to
    O^T after the PV accumulation (rank-1 ones matmul broadcasts 1/d)
  - PV: O^T[c, q] += V[j, :]^T P^T[j, q] accumulated in PSUM over all
    32 j-tiles (single pass)
  - final wo projection + bias + residual of q-block N deferred into
    q-block N+1's PE stream to fill the scores->exp->PV latency bubble
"""

from contextlib import ExitStack

import concourse.bass as bass
import concourse.tile as tile
from concourse import mybir
from concourse.masks import make_identity

F32 = mybir.dt.float32
F32R = mybir.dt.float32r
BF16 = mybir.dt.bfloat16
AX = mybir.AxisListType
OP = mybir.AluOpType
ACTF = mybir.ActivationFunctionType

C = 512
NCH = 4  # channel chunks of 128
GPC = 8  # groups per 128-channel chunk (16 channels per group)
EPS = 1e-5


def build(nc: bass.Bass, HW: int = 4096):
    SCALE_Q = float(C) ** (-0.5)
    NJB = HW // 512      # 512-col j/q blocks
    NQB = HW // 512
    NJT = HW // 128      # 128-col j tiles
    GN_N = 16 * HW       # elements per group

    x = nc.dram_tensor("x", [C, HW], F32, kind="ExternalInput")
    gn_w = nc.dram_tensor("gn_w", [C], F32, kind="ExternalInput")
    gn_b = nc.dram_tensor("gn_b", [C], F32, kind="ExternalInput")
    wq = nc.dram_tensor("wq", [C, C], F32, kind="ExternalInput")
    bq = nc.dram_tensor("bq", [C], F32, kind="ExternalInput")
    wk = nc.dram_tensor("wk", [C, C], F32, kind="ExternalInput")
    bk = nc.dram_tensor("bk", [C], F32, kind="ExternalInput")
    wv = nc.dram_tensor("wv", [C, C], F32, kind="ExternalInput")
    bv = nc.dram_tensor("bv", [C], F32, kind="ExternalInput")
    wo = nc.dram_tensor("wo", [C, C], F32, kind="ExternalInput")
    bo = nc.dram_tensor("bo", [C], F32, kind="ExternalInput")
    out = nc.dram_tensor("out", [C, HW], F32, kind="ExternalOutput")

    x_r = x.rearrange("(c p) q -> p c q", p=128)
    out_r = out.rearrange("(c p) q -> p c q", p=128)

    with tile.TileContext(nc) as tc, ExitStack() as ctx:
        pconst = ctx.enter_context(tc.tile_pool(name="const", bufs=1))
        ppersist = ctx.enter_context(tc.tile_pool(name="persist", bufs=1))
        pstream = ctx.enter_context(tc.tile_pool(name="stream", bufs=2))

        # ---- constants ----
        identity = pconst.tile([128, 128], F32, tag="ident")
        make_identity(nc, identity[:])
        ones128_f = pconst.tile([128, 1], F32, tag="ones128_f")
        nc.gpsimd.memset(ones128_f[:], 1.0)
        ones128 = pconst.tile([128, 1], F32R, tag="ones128")
        nc.vector.tensor_copy(ones128[:], ones128_f[:])
        ones1_f = pconst.tile([1, 128], F32, tag="ones1_f")
        nc.gpsimd.memset(ones1_f[:], 1.0)
        ones1 = pconst.tile([1, 128], F32R, tag="ones1")
        nc.vector.tensor_copy(ones1[:], ones1_f[:])
        # group indicator matrices: ind8[c, g] = e8[g, c] = (c // 16 == g)
        ind8_f = pconst.tile([128, GPC], F32, tag="ind8_f")
        nc.gpsimd.memset(ind8_f[:], 1.0)
        nc.gpsimd.affine_select(
            out=ind8_f[:], in_=ind8_f[:], compare_op=OP.is_ge, fill=0.0,
            base=0, channel_multiplier=1, pattern=[[-16, GPC]],
        )
        nc.gpsimd.affine_select(
            out=ind8_f[:], in_=ind8_f[:], compare_op=OP.is_ge, fill=0.0,
            base=15, channel_multiplier=-1, pattern=[[16, GPC]],
        )
        ind8 = pconst.tile([128, GPC], F32R, tag="ind8")
        nc.vector.tensor_copy(ind8[:], ind8_f[:])
        e8_f = pconst.tile([GPC, 128], F32, tag="e8_f")
        nc.gpsimd.memset(e8_f[:], 1.0)
        nc.gpsimd.affine_select(
            out=e8_f[:], in_=e8_f[:], compare_op=OP.is_ge, fill=0.0,
            base=0, channel_multiplier=-16, pattern=[[1, 128]],
        )
        nc.gpsimd.affine_select(
            out=e8_f[:], in_=e8_f[:], compare_op=OP.is_ge, fill=0.0,
            base=15, channel_multiplier=16, pattern=[[-1, 128]],
        )
        e8 = pconst.tile([GPC, 128], F32R, tag="e8")
        nc.vector.tensor_copy(e8[:], e8_f[:])

        gnw4 = pconst.tile([128, NCH], F32, tag="gnw4")
        gnb4 = pconst.tile([128, NCH], F32, tag="gnb4")
        bq4 = pconst.tile([128, NCH], F32, tag="bq4")
        bqs4 = pconst.tile([128, NCH], F32, tag="bqs4")
        bk4 = pconst.tile([128, NCH], F32, tag="bk4")
        bo4 = pconst.tile([128, NCH], F32, tag="bo4")
        for t, src in ((gnw4, gn_w), (gnb4, gn_b), (bq4, bq), (bk4, bk), (bo4, bo)):
            nc.sync.dma_start(out=t[:], in_=src.rearrange("(c p) -> p c", p=128))
        nc.vector.tensor_scalar_mul(bqs4[:], bq4[:], SCALE_Q)
        bv_row = pconst.tile([1, C], F32, tag="bv_row")
        nc.sync.dma_start(out=bv_row[:], in_=bv.rearrange("(a i) -> a i", a=1))
        bv_row_r = pconst.tile([1, C], F32R, tag="bv_row_r")
        nc.vector.tensor_copy(bv_row_r[:], bv_row[:])
        bv_bcast = pconst.tile([128, C], F32, tag="bv_bcast")

        eps_t = pconst.tile([GPC, 1], F32, tag="eps_t")
        nc.gpsimd.memset(eps_t[:], EPS)
        sum_cols = pconst.tile([128, NCH, NJB], F32, tag="sum_cols")
        sq_cols = pconst.tile([128, NCH, NJB], F32, tag="sq_cols")
        ch_stats_r = pconst.tile([128, NCH, 2], F32R, tag="ch_stats_r")
        scale4 = pconst.tile([128, NCH], F32, tag="scale4")
        shift4 = pconst.tile([128, NCH], F32, tag="shift4")

        # ---- persistent tensors (all resident, no spills) ----
        x_sb = ppersist.tile([128, NCH, HW], BF16, tag="x_sb")
        qt = ppersist.tile([128, NCH, HW], BF16, tag="qt")
        kt = ppersist.tile([128, NCH, HW], BF16, tag="kt")
        vt = ppersist.tile([128, NJT, C], BF16, tag="vt")
        woT = ppersist.tile([128, NCH, C], BF16, tag="woT")

        # ---- phase A ----
        with tc.tile_pool(name="wqkv", bufs=1) as pwqkv:
            wqT = pwqkv.tile([128, NCH, C], BF16, tag="wqT")
            wkT = pwqkv.tile([128, NCH, C], BF16, tag="wkT")
            wvT = pwqkv.tile([128, NCH, C], BF16, tag="wvT")

            with tc.tile_pool(name="psA", bufs=1, space="PSUM") as psA:
                with tc.tile_pool(name="scrA", bufs=2) as pscr:
                    # ---- pass 1: GN statistics + bf16 copy of x into SBUF ----
                    for jb in range(NJB):
                        x_in = pstream.tile([128, NCH, 512], F32, tag="xin")
                        nc.sync.dma_start(
                            out=x_in[:], in_=x_r[:, :, 512 * jb : 512 * (jb + 1)]
                        )
                        for ci in range(NCH):
                            # copy to resident bf16 x and row-sum in one ACT pass
                            nc.scalar.activation(
                                x_sb[:, ci, 512 * jb : 512 * (jb + 1)],
                                x_in[:, ci, :],
                                ACTF.Identity,
                                accum_out=sum_cols[:, ci, jb : jb + 1],
                            )
                            xsq = pscr.tile([128, 512], F32, tag="xsq")
                            nc.scalar.activation(
                                xsq[:],
                                x_in[:, ci, :],
                                ACTF.Square,
                                accum_out=sq_cols[:, ci, jb : jb + 1],
                            )
                    # weight transposes: wT[:, ci, co*128:..] = W[co blk, ci blk].T
                    with tc.tile_pool(name="raw", bufs=2) as praw:
                        for w_ext, wT in ((wq, wqT), (wk, wkT), (wv, wvT), (wo, woT)):
                            raw = praw.tile([128, NCH, C], F32, tag="raw")
                            nc.sync.dma_start(
                                out=raw[:], in_=w_ext.rearrange("(c p) i -> p c i", p=128)
                            )
                            for co in range(NCH):
                                for ci in range(NCH):
                                    ps = psA.tile([128, 128], F32, tag="m", bufs=4)
                                    nc.tensor.transpose(
                                        ps[:],
                                        raw[:, co, 128 * ci : 128 * (ci + 1)],
                                        identity[:],
                                    )
                                    nc.scalar.activation(
                                        wT[:, ci, 128 * co : 128 * (co + 1)],
                                        ps[:],
                                        ACTF.Identity,
                                    )
                        # bv broadcast tile (rank-1 matmul)
                        psbv = psA.tile([128, C], F32, tag="m", bufs=4)
                        nc.tensor.matmul(
                            psbv[:], ones1[:], bv_row_r[:], start=True, stop=True
                        )
                        nc.scalar.activation(bv_bcast[:], psbv[:], ACTF.Identity)
                    # combine stats -> per-channel scale/shift
                    for ci in range(NCH):
                        with nc.allow_low_precision(
                            reason="f32r rounding of GN sums is ~2^-11 relative"
                        ):
                            nc.vector.reduce_sum(
                                ch_stats_r[:, ci, 0:1], sum_cols[:, ci, :], axis=AX.X
                            )
                            nc.vector.reduce_sum(
                                ch_stats_r[:, ci, 1:2], sq_cols[:, ci, :], axis=AX.X
                            )
                        psg = psA.tile([GPC, 2], F32, tag="t", bufs=2)
                        nc.tensor.matmul(
                            psg[:], ind8[:], ch_stats_r[:, ci, :], start=True, stop=True
                        )
                        mean = pscr.tile([GPC, 1], F32, tag="st_mean")
                        ex2 = pscr.tile([GPC, 1], F32, tag="st_ex2")
                        nc.vector.tensor_scalar_mul(mean[:], psg[:, 0:1], 1.0 / GN_N)
                        nc.vector.tensor_scalar_mul(ex2[:], psg[:, 1:2], 1.0 / GN_N)
                        var = pscr.tile([GPC, 1], F32, tag="st_var")
                        nc.vector.tensor_mul(var[:], mean[:], mean[:])
                        nc.vector.tensor_sub(var[:], ex2[:], var[:])
                        std = pscr.tile([GPC, 1], F32, tag="st_std")
                        nc.scalar.activation(std[:], var[:], ACTF.Sqrt, bias=eps_t[:])
                        rstd = pscr.tile([GPC, 1], F32, tag="st_rstd")
                        nc.vector.reciprocal(rstd[:], std[:])
                        st2 = pscr.tile([GPC, 2], F32R, tag="st2")
                        nc.vector.tensor_copy(st2[:, 0:1], rstd[:])
                        nc.vector.tensor_copy(st2[:, 1:2], mean[:])
                        pse = psA.tile([128, 2], F32, tag="t", bufs=2)
                        nc.tensor.matmul(pse[:], e8[:], st2[:], start=True, stop=True)
                        # scale = rstd * gamma ; shift = beta - mean * scale
                        nc.vector.tensor_mul(
                            scale4[:, ci : ci + 1], pse[:, 0:1], gnw4[:, ci : ci + 1]
                        )
                        tmp = pscr.tile([128, 1], F32, tag="st_tmp")
                        nc.vector.tensor_mul(
                            tmp[:], pse[:, 1:2], scale4[:, ci : ci + 1]
                        )
                        nc.vector.tensor_sub(
                            shift4[:, ci : ci + 1], gnb4[:, ci : ci + 1], tmp[:]
                        )

                    # ---- pass 2: GN apply + Q/K/V projections (from SBUF x) ----
                    for jb in range(NJB):
                        hn = pscr.tile([128, NCH, 512], BF16, tag="hn")
                        for ci in range(NCH):
                            nc.scalar.activation(
                                hn[:, ci, :],
                                x_sb[:, ci, 512 * jb : 512 * (jb + 1)],
                                ACTF.Identity,
                                scale=scale4[:, ci : ci + 1],
                                bias=shift4[:, ci : ci + 1],
                            )
                        # Q (scaled by 1/sqrt(C)) and K, channel-major
                        for wT, dst, bias_ap, scl in (
                            (wqT, qt, bqs4, SCALE_Q),
                            (wkT, kt, bk4, 1.0),
                        ):
                            for co in range(NCH):
                                psq = psA.tile([128, 512], F32, tag="m", bufs=4)
                                for ci in range(NCH):
                                    nc.tensor.matmul(
                                        psq[:],
                                        wT[:, ci, 128 * co : 128 * (co + 1)],
                                        hn[:, ci, :],
                                        start=(ci == 0),
                                        stop=(ci == NCH - 1),
                                    )
                                nc.scalar.activation(
                                    dst[:, co, 512 * jb : 512 * (jb + 1)],
                                    psq[:],
                                    ACTF.Identity,
                                    scale=scl,
                                    bias=bias_ap[:, co : co + 1],
                                )
                        # V[j, c] per j-subtile, token-major
                        for jtl in range(4):
                            psv = psA.tile([128, 512], F32, tag="m", bufs=4)
                            for ci in range(NCH):
                                nc.tensor.matmul(
                                    psv[:],
                                    hn[:, ci, 128 * jtl : 128 * (jtl + 1)],
                                    wvT[:, ci, :],
                                    start=(ci == 0),
                                    stop=(ci == NCH - 1),
                                )
                            nc.vector.tensor_add(
                                vt[:, 4 * jb + jtl, :], psv[:], bv_bcast[:]
                            )

        # ---- phase B: single pass over all q-blocks, full K/V resident ----
        with (
            tc.tile_pool(name="poolB", bufs=1) as pB,
            tc.tile_pool(name="psB", bufs=1, space="PSUM") as psB,
        ):
            pending = None

            def emit_epilogue(p):
                # deferred final projection + bias + residual for a prior
                # q-block; spliced into the next q-block's PE stream so it
                # fills the scores->exp->PV latency bubble
                e_qb, e_osb, e_rbc = p
                outs = pB.tile([128, NCH, 512], F32, tag="outs", bufs=2)
                for co in range(NCH):
                    psf = psB.tile([128, 512], F32, tag="f", bufs=2)
                    for cc in range(NCH):
                        nc.tensor.matmul(
                            psf[:],
                            woT[:, cc, 128 * co : 128 * (co + 1)],
                            e_osb[:, cc, :],
                            start=(cc == 0),
                            stop=(cc == NCH - 1),
                        )
                    xb = pB.tile([128, 512], F32, tag="xb", bufs=2)
                    nc.scalar.activation(
                        xb[:],
                        x_sb[:, co, 512 * e_qb : 512 * (e_qb + 1)],
                        ACTF.Identity,
                        bias=bo4[:, co : co + 1],
                    )
                    nc.vector.tensor_mul(outs[:, co, :], psf[:], e_rbc[:])
                    nc.vector.tensor_add(
                        outs[:, co, :], outs[:, co, :], xb[:]
                    )
                nc.sync.dma_start(
                    out=out_r[:, :, 512 * e_qb : 512 * (e_qb + 1)], in_=outs[:]
                )

            for qb in range(NQB):
                # den accumulated in f32 (DVE converts bf16 exp tiles on read);
                # stored as f32r so the ones-matmul can consume it directly
                den = pB.tile([128, 512], F32R, tag="den", bufs=2)
                pso = [
                    psB.tile([128, 512], F32, tag="o", bufs=4, name="pso")
                    for _ in range(NCH)
                ]
                for jt in range(NJT):
                    pss = psB.tile([128, 512], F32, tag="s", bufs=2)
                    for ci in range(NCH):
                        nc.tensor.matmul(
                            pss[:],
                            kt[:, ci, 128 * jt : 128 * (jt + 1)],
                            qt[:, ci, 512 * qb : 512 * (qb + 1)],
                            start=(ci == 0),
                            stop=(ci == NCH - 1),
                        )
                    if jt == 0 and pending is not None:
                        emit_epilogue(pending)
                        pending = None
                    pt = pB.tile([128, 512], BF16, tag="pt", bufs=3)
                    nc.scalar.activation(pt[:], pss[:], ACTF.Exp)
                    if jt == 0:
                        nc.vector.tensor_copy(den[:], pt[:])
                    else:
                        nc.vector.tensor_add(den[:], den[:].bitcast(F32), pt[:])
                    for cc in range(NCH):
                        nc.tensor.matmul(
                            pso[cc][:],
                            vt[:, jt, 128 * cc : 128 * (cc + 1)],
                            pt[:],
                            start=(jt == 0),
                            stop=(jt == NJT - 1),
                        )
                # partition-sum of den, broadcast of 1/den
                psd = psB.tile([1, 512], F32, tag="f", bufs=2)
                nc.tensor.matmul(psd[:], ones128[:], den[:], start=True, stop=True)
                dtot = pB.tile([1, 512], F32R, tag="dtot", bufs=2)
                nc.vector.tensor_copy(dtot[:], psd[:])
                psb = psB.tile([128, 512], F32, tag="f", bufs=2)
                nc.tensor.matmul(psb[:], ones1[:], dtot[:], start=True, stop=True)
                rbc = pB.tile([128, 512], F32, tag="rbc", bufs=2)
                nc.vector.reciprocal(rbc[:], psb[:])
                # drain unnormalized O^T to SBUF (frees the pso banks)
                osb = pB.tile([128, NCH, 512], BF16, tag="osb", bufs=2)
                for cc in range(NCH):
                    nc.scalar.activation(osb[:, cc, :], pso[cc][:], ACTF.Identity)
                pending = (qb, osb, rbc)
            if pending is not None:
                emit_epilogue(pending)
                pending = None
    return nc


# ---------------------------------------------------------------------------
# Walrus in this container rejects instructions carrying more than ~2
# sync-wait commands ("Too many sync wait commands").  Hoist excess on_wait
# entries onto nofuse NOPs placed immediately before the instruction on the
# same engine (engines issue in-order, so blocking on the NOP first is
# equivalent).
def split_sync_waits(nc, max_waits=1):
    n_split = 0
    for bb in nc.main_func.blocks:
        insts = bb.instructions
        out = []
        for inst in insts:
            si = inst.sync_info
            if si is not None and si.on_wait is not None and len(si.on_wait) > max_waits:
                waits = list(si.on_wait)
                keep = waits[-max_waits:]
                extra = waits[:-max_waits]
                for i in range(0, len(extra), max_waits):
                    chunk = extra[i : i + max_waits]
                    nop = mybir.InstNoOp(
                        name=f"{inst.name}-sw{i}",
                        sync_info=mybir.SyncInfo(on_wait=chunk, on_update=[]),
                        bass_nofuse=True,
                        engine=inst.engine,
                    )
                    out.append(nop)
                    n_split += 1
                inst.sync_info = mybir.SyncInfo(
                    on_wait=keep, on_update=list(si.on_update or [])
                )
            out.append(inst)
        bb.instructions = out
    return n_split


B, H, W = 8, 64, 64
HW = H * W
N_CORES = 8
_CACHE = {}


def _get_nc():
    if "nc" not in _CACHE:
        nc = bass.Bass()
        build(nc, HW=HW)
        split_sync_waits(nc)
        _CACHE["nc"] = nc
    return _CACHE["nc"]


def _in_maps(inputs):
    import numpy as np
    arrs = {k: np.ascontiguousarray(np.asarray(v, dtype=np.float32)) for k, v in inputs.items()}
    x = arrs.pop("x").reshape(B, C, HW)
    return [{"x": x[i], **arrs} for i in range(N_CORES)]


def kernel(**inputs):
    import numpy as np
    from concourse.bass_utils import run_bass_kernel_spmd

    nc = _get_nc()
    res = run_bass_kernel_spmd(nc, _in_maps(inputs), list(range(N_CORES)))
    out = np.stack([res.results[i]["out"] for i in range(N_CORES)])
    return out.reshape(B, C, H, W).astype(np.float32)


def kernel_traced(**inputs):
    """Like kernel() but with NTFF profiling; returns (output, BassKernelResults)."""
    import numpy as np
    from concourse.bass_utils import run_bass_kernel_spmd

    nc = _get_nc()
    res = run_bass_kernel_spmd(
        nc, _in_maps(inputs), list(range(N_CORES)), trace=True
    )
    out = np.stack([res.results[i]["out"] for i in range(N_CORES)])
    return out.reshape(B, C, H, W).astype(np.float32), res


# revision 16
# speedup vs baseline: 1.3008x; 1.1410x over previous
import sys

for _p in ("/opt/trn_rl_repo",):
    if _p not in sys.path:
        sys.path.append(_p)

"""AttnBlock (GroupNorm + single-head self-attention + residual) Bass/Tile
kernel for one NeuronCore (one batch sample), channel-major layout.

Per-core problem:  x [C=512, HW] f32
  hn = groupnorm(x, 32 groups, eps=1e-5) * gn_w + gn_b
  q/k/v = 1x1 conv (C x C) on tokens;  scores = (q k^T) / sqrt(C)
  attn = softmax(scores);  o = attn @ v;  out = x + (o @ wo^T + bo)

Layout strategy (all big matmuls in bf16: same 1 col/cycle PE rate as
f32r but enables fast-weight-load so LDWEIGHTS hides behind streaming;
bf16 also halves SBUF so Qt/Kt/V and x all stay resident -> single-pass
phase B with zero DRAM spills):
  - x kept in SBUF as bf16 after the stats pass (residual + pass-2 reads)
  - hn, Qt, Kt channel-major [c, hw];  V token-major [hw, c]
  - scores computed transposed St[j, q] = sum_c Kt[c,j] Qt[c,q]
  - exp via ACT, no max subtraction (scores ~N(0,1) by construction)
  - softmax denominator: elementwise accumulate exp tiles on DVE, then a
    ones-vector matmul for the partition sum; normalization applied to
    O^T after the PV accumulation (rank-1 ones matmul broadcasts 1/d)
  - PV: O^T[c, q] += V[j, :]^T P^T[j, q] accumulated in PSUM over all
    32 j-tiles (single pass)
  - final wo projection + bias + residual of q-block N deferred into
    q-block N+1's PE stream to fill the scores->exp->PV latency bubble
"""

from contextlib import ExitStack

import concourse.bass as bass
import concourse.tile as tile
from concourse import mybir
from concourse.masks import make_identity

F32 = mybir.dt.float32
F32R = mybir.dt.float32r
BF16 = mybir.dt.bfloat16
AX = mybir.AxisListType
OP = mybir.AluOpType
ACTF = mybir.ActivationFunctionType

C = 512
NCH = 4  # channel chunks of 128
GPC = 8  # groups per 128-channel chunk (16 channels per group)
EPS = 1e-5


def build(nc: bass.Bass, HW: int = 4096):
    SCALE_Q = float(C) ** (-0.5)
    NJB = HW // 512      # 512-col j/q blocks
    NQB = HW // 512
    NJT = HW // 128      # 128-col j tiles
    GN_N = 16 * HW       # elements per group

    x = nc.dram_tensor("x", [C, HW], F32, kind="ExternalInput")
    gn_w = nc.dram_tensor("gn_w", [C], F32, kind="ExternalInput")
    gn_b = nc.dram_tensor("gn_b", [C], F32, kind="ExternalInput")
    wq = nc.dram_tensor("wq", [C, C], F32, kind="ExternalInput")
    bq = nc.dram_tensor("bq", [C], F32, kind="ExternalInput")
    wk = nc.dram_tensor("wk", [C, C], F32, kind="ExternalInput")
    bk = nc.dram_tensor("bk", [C], F32, kind="ExternalInput")
    wv = nc.dram_tensor("wv", [C, C], F32, kind="ExternalInput")
    bv = nc.dram_tensor("bv", [C], F32, kind="ExternalInput")
    wo = nc.dram_tensor("wo", [C, C], F32, kind="ExternalInput")
    bo = nc.dram_tensor("bo", [C], F32, kind="ExternalInput")
    out = nc.dram_tensor("out", [C, HW], F32, kind="ExternalOutput")

    x_r = x.rearrange("(c p) q -> p c q", p=128)
    out_r = out.rearrange("(c p) q -> p c q", p=128)

    with tile.TileContext(nc) as tc, ExitStack() as ctx:
        pconst = ctx.enter_context(tc.tile_pool(name="const", bufs=1))
        ppersist = ctx.enter_context(tc.tile_pool(name="persist", bufs=1))

        # ---- constants ----
        identity = pconst.tile([128, 128], F32, tag="ident")
        make_identity(nc, identity[:])
        ones128_f = pconst.tile([128, 1], F32, tag="ones128_f")
        nc.gpsimd.memset(ones128_f[:], 1.0)
        ones128 = pconst.tile([128, 1], F32R, tag="ones128")
        nc.vector.tensor_copy(ones128[:], ones128_f[:])
        ones1_f = pconst.tile([1, 128], F32, tag="ones1_f")
        nc.gpsimd.memset(ones1_f[:], 1.0)
        ones1 = pconst.tile([1, 128], F32R, tag="ones1")
        nc.vector.tensor_copy(ones1[:], ones1_f[:])
        # group indicator matrices: ind8[c, g] = e8[g, c] = (c // 16 == g)
        ind8_f = pconst.tile([128, GPC], F32, tag="ind8_f")
        nc.gpsimd.memset(ind8_f[:], 1.0)
        nc.gpsimd.affine_select(
            out=ind8_f[:], in_=ind8_f[:], compare_op=OP.is_ge, fill=0.0,
            base=0, channel_multiplier=1, pattern=[[-16, GPC]],
        )
        nc.gpsimd.affine_select(
            out=ind8_f[:], in_=ind8_f[:], compare_op=OP.is_ge, fill=0.0,
            base=15, channel_multiplier=-1, pattern=[[16, GPC]],
        )
        ind8 = pconst.tile([128, GPC], F32R, tag="ind8")
        nc.vector.tensor_copy(ind8[:], ind8_f[:])
        e8_f = pconst.tile([GPC, 128], F32, tag="e8_f")
        nc.gpsimd.memset(e8_f[:], 1.0)
        nc.gpsimd.affine_select(
            out=e8_f[:], in_=e8_f[:], compare_op=OP.is_ge, fill=0.0,
            base=0, channel_multiplier=-16, pattern=[[1, 128]],
        )
        nc.gpsimd.affine_select(
            out=e8_f[:], in_=e8_f[:], compare_op=OP.is_ge, fill=0.0,
            base=15, channel_multiplier=16, pattern=[[-1, 128]],
        )
        e8 = pconst.tile([GPC, 128], F32R, tag="e8")
        nc.vector.tensor_copy(e8[:], e8_f[:])

        gnw4 = pconst.tile([128, NCH], F32, tag="gnw4")
        gnb4 = pconst.tile([128, NCH], F32, tag="gnb4")
        bq4 = pconst.tile([128, NCH], F32, tag="bq4")
        bqs4 = pconst.tile([128, NCH], F32, tag="bqs4")
        bk4 = pconst.tile([128, NCH], F32, tag="bk4")
        bo4 = pconst.tile([128, NCH], F32, tag="bo4")
        for t, src in ((gnw4, gn_w), (gnb4, gn_b), (bq4, bq), (bk4, bk), (bo4, bo)):
            nc.sync.dma_start(out=t[:], in_=src.rearrange("(c p) -> p c", p=128))
        nc.vector.tensor_scalar_mul(bqs4[:], bq4[:], SCALE_Q)
        bv_row = pconst.tile([1, C], F32, tag="bv_row")
        nc.sync.dma_start(out=bv_row[:], in_=bv.rearrange("(a i) -> a i", a=1))
        bv_row_r = pconst.tile([1, C], F32R, tag="bv_row_r")
        nc.vector.tensor_copy(bv_row_r[:], bv_row[:])
        bv_bcast = pconst.tile([128, C], F32, tag="bv_bcast")

        eps_t = pconst.tile([GPC, 1], F32, tag="eps_t")
        nc.gpsimd.memset(eps_t[:], EPS)
        bnst = pconst.tile([128, NCH, NJB, 6], F32, tag="bnst")
        cmv = pconst.tile([128, NCH, 2], F32, tag="cmv")
        ch_stats_r = pconst.tile([128, NCH, 2], F32R, tag="ch_stats_r")
        scale4 = pconst.tile([128, NCH], F32, tag="scale4")
        shift4 = pconst.tile([128, NCH], F32, tag="shift4")

        # ---- persistent tensors (all resident, no spills) ----
        x_sb = ppersist.tile([128, NCH, HW], BF16, tag="x_sb")
        qt = ppersist.tile([128, NCH, HW], BF16, tag="qt")
        kt = ppersist.tile([128, NCH, HW], BF16, tag="kt")
        vt = ppersist.tile([128, NJT, C], BF16, tag="vt")
        woT = ppersist.tile([128, NCH, C], BF16, tag="woT")

        # ---- phase A ----
        with tc.tile_pool(name="wqkv", bufs=1) as pwqkv, \
             tc.tile_pool(name="stream", bufs=2) as pstream:
            wqT = pwqkv.tile([128, NCH, C], BF16, tag="wqT")
            wkT = pwqkv.tile([128, NCH, C], BF16, tag="wkT")
            wvT = pwqkv.tile([128, NCH, C], BF16, tag="wvT")

            with tc.tile_pool(name="psA", bufs=1, space="PSUM") as psA:
                with tc.tile_pool(name="scrA", bufs=2) as pscr, \
                     tc.tile_pool(name="raw", bufs=2) as praw:
                    # weight DMAs issued first so transposes can fill the
                    # stats pass on the PE
                    raws = []
                    for w_ext in (wq, wk, wv, wo):
                        raw = praw.tile([128, NCH, C], F32, tag="raw")
                        nc.sync.dma_start(
                            out=raw[:], in_=w_ext.rearrange("(c p) i -> p c i", p=128)
                        )
                        raws.append(raw)
                    # ---- pass 1: GN statistics + bf16 copy of x into SBUF ----
                    # all elementwise work on DVE (bn_stats one-pass mean/var)
                    # so the ACT queue stays free for the transpose PSUM drains
                    for jb in range(NJB):
                        x_in = pstream.tile([128, NCH, 512], F32, tag="xin")
                        nc.sync.dma_start(
                            out=x_in[:], in_=x_r[:, :, 512 * jb : 512 * (jb + 1)]
                        )
                        for ci in range(NCH):
                            nc.vector.tensor_copy(
                                x_sb[:, ci, 512 * jb : 512 * (jb + 1)],
                                x_in[:, ci, :],
                            )
                            nc.vector.bn_stats(
                                bnst[:, ci, jb, :], x_in[:, ci, :]
                            )
                    # weight transposes: wT[:, ci, co*128:..] = W[co blk, ci blk].T
                    if True:
                        for raw, wT in zip(raws, (wqT, wkT, wvT, woT)):
                            for co in range(NCH):
                                for ci in range(NCH):
                                    ps = psA.tile([128, 128], F32, tag="m", bufs=4)
                                    nc.tensor.transpose(
                                        ps[:],
                                        raw[:, co, 128 * ci : 128 * (ci + 1)],
                                        identity[:],
                                    )
                                    nc.scalar.activation(
                                        wT[:, ci, 128 * co : 128 * (co + 1)],
                                        ps[:],
                                        ACTF.Identity,
                                    )
                        # bv broadcast tile (rank-1 matmul)
                        psbv = psA.tile([128, C], F32, tag="m", bufs=4)
                        nc.tensor.matmul(
                            psbv[:], ones1[:], bv_row_r[:], start=True, stop=True
                        )
                        nc.scalar.activation(bv_bcast[:], psbv[:], ACTF.Identity)
                    # combine stats -> per-channel scale/shift
                    for ci in range(NCH):
                        # per-channel (mean, E[x^2]) from bn chunk stats
                        nc.vector.bn_aggr(cmv[:, ci, :], bnst[:, ci, :, :])
                        m2 = pscr.tile([128, 1], F32, tag="st_m2")
                        nc.vector.tensor_mul(
                            m2[:], cmv[:, ci, 0:1], cmv[:, ci, 0:1]
                        )
                        nc.vector.tensor_copy(
                            ch_stats_r[:, ci, 0:1], cmv[:, ci, 0:1]
                        )
                        nc.vector.tensor_add(
                            ch_stats_r[:, ci, 1:2], cmv[:, ci, 1:2], m2[:]
                        )
                        psg = psA.tile([GPC, 2], F32, tag="t", bufs=2)
                        nc.tensor.matmul(
                            psg[:], ind8[:], ch_stats_r[:, ci, :], start=True, stop=True
                        )
                        mean = pscr.tile([GPC, 1], F32, tag="st_mean")
                        ex2 = pscr.tile([GPC, 1], F32, tag="st_ex2")
                        nc.vector.tensor_scalar_mul(mean[:], psg[:, 0:1], 1.0 / 16.0)
                        nc.vector.tensor_scalar_mul(ex2[:], psg[:, 1:2], 1.0 / 16.0)
                        var = pscr.tile([GPC, 1], F32, tag="st_var")
                        nc.vector.tensor_mul(var[:], mean[:], mean[:])
                        nc.vector.tensor_sub(var[:], ex2[:], var[:])
                        std = pscr.tile([GPC, 1], F32, tag="st_std")
                        nc.scalar.activation(std[:], var[:], ACTF.Sqrt, bias=eps_t[:])
                        rstd = pscr.tile([GPC, 1], F32, tag="st_rstd")
                        nc.vector.reciprocal(rstd[:], std[:])
                        st2 = pscr.tile([GPC, 2], F32R, tag="st2")
                        nc.vector.tensor_copy(st2[:, 0:1], rstd[:])
                        nc.vector.tensor_copy(st2[:, 1:2], mean[:])
                        pse = psA.tile([128, 2], F32, tag="t", bufs=2)
                        nc.tensor.matmul(pse[:], e8[:], st2[:], start=True, stop=True)
                        # scale = rstd * gamma ; shift = beta - mean * scale
                        nc.vector.tensor_mul(
                            scale4[:, ci : ci + 1], pse[:, 0:1], gnw4[:, ci : ci + 1]
                        )
                        tmp = pscr.tile([128, 1], F32, tag="st_tmp")
                        nc.vector.tensor_mul(
                            tmp[:], pse[:, 1:2], scale4[:, ci : ci + 1]
                        )
                        nc.vector.tensor_sub(
                            shift4[:, ci : ci + 1], gnb4[:, ci : ci + 1], tmp[:]
                        )

                    # ---- pass 2: GN apply + Q/K/V projections (from SBUF x) ----
                    for jb in range(NJB):
                        hn = pscr.tile([128, NCH, 512], BF16, tag="hn")
                        for ci in range(NCH):
                            nc.scalar.activation(
                                hn[:, ci, :],
                                x_sb[:, ci, 512 * jb : 512 * (jb + 1)],
                                ACTF.Identity,
                                scale=scale4[:, ci : ci + 1],
                                bias=shift4[:, ci : ci + 1],
                            )
                        # Q (scaled by 1/sqrt(C)) and K, channel-major
                        for wT, dst, bias_ap, scl in (
                            (wqT, qt, bqs4, SCALE_Q),
                            (wkT, kt, bk4, 1.0),
                        ):
                            for co in range(NCH):
                                psq = psA.tile([128, 512], F32, tag="m", bufs=4)
                                for ci in range(NCH):
                                    nc.tensor.matmul(
                                        psq[:],
                                        wT[:, ci, 128 * co : 128 * (co + 1)],
                                        hn[:, ci, :],
                                        start=(ci == 0),
                                        stop=(ci == NCH - 1),
                                    )
                                nc.scalar.activation(
                                    dst[:, co, 512 * jb : 512 * (jb + 1)],
                                    psq[:],
                                    ACTF.Identity,
                                    scale=scl,
                                    bias=bias_ap[:, co : co + 1],
                                )
                        # V[j, c] per j-subtile, token-major
                        for jtl in range(4):
                            psv = psA.tile([128, 512], F32, tag="m", bufs=4)
                            for ci in range(NCH):
                                nc.tensor.matmul(
                                    psv[:],
                                    hn[:, ci, 128 * jtl : 128 * (jtl + 1)],
                                    wvT[:, ci, :],
                                    start=(ci == 0),
                                    stop=(ci == NCH - 1),
                                )
                            nc.vector.tensor_add(
                                vt[:, 4 * jb + jtl, :], psv[:], bv_bcast[:]
                            )

        # ---- phase B: single pass over all q-blocks, full K/V resident ----
        with (
            tc.tile_pool(name="poolB", bufs=1) as pB,
            tc.tile_pool(name="psB", bufs=1, space="PSUM") as psB,
        ):
            pending = None

            def emit_epilogue(p):
                # deferred final projection + bias + residual for a prior
                # q-block; spliced into the next q-block's PE stream so it
                # fills the scores->exp->PV latency bubble
                e_qb, e_osb, e_rbc, e_xb = p
                outs = pB.tile([128, NCH, 512], F32, tag="outs", bufs=2)
                for co in range(NCH):
                    psf = psB.tile([128, 512], F32, tag="f", bufs=2)
                    for cc in range(NCH):
                        nc.tensor.matmul(
                            psf[:],
                            woT[:, cc, 128 * co : 128 * (co + 1)],
                            e_osb[:, cc, :],
                            start=(cc == 0),
                            stop=(cc == NCH - 1),
                        )
                    nc.vector.tensor_mul(outs[:, co, :], psf[:], e_rbc[:])
                    nc.vector.tensor_add(
                        outs[:, co, :], outs[:, co, :], e_xb[:, co, :]
                    )
                nc.sync.dma_start(
                    out=out_r[:, :, 512 * e_qb : 512 * (e_qb + 1)], in_=outs[:]
                )

            for qb in range(NQB):
                # residual + output bias staged early on DVE (off the ACT
                # queue, consumed by the deferred epilogue next q-block)
                xb = pB.tile([128, NCH, 512], BF16, tag="xb", bufs=2)
                for co in range(NCH):
                    nc.vector.tensor_scalar_add(
                        xb[:, co, :],
                        x_sb[:, co, 512 * qb : 512 * (qb + 1)],
                        bo4[:, co : co + 1],
                    )
                # den accumulated in f32 (DVE converts bf16 exp tiles on read);
                # stored as f32r so the ones-matmul can consume it directly
                den = pB.tile([128, 512], F32R, tag="den", bufs=2)
                pso = [
                    psB.tile([128, 512], F32, tag="o", bufs=4, name="pso")
                    for _ in range(NCH)
                ]
                for jt in range(NJT):
                    pss = psB.tile([128, 512], F32, tag="s", bufs=2)
                    for ci in range(NCH):
                        nc.tensor.matmul(
                            pss[:],
                            kt[:, ci, 128 * jt : 128 * (jt + 1)],
                            qt[:, ci, 512 * qb : 512 * (qb + 1)],
                            start=(ci == 0),
                            stop=(ci == NCH - 1),
                        )
                    if jt == 0 and pending is not None:
                        emit_epilogue(pending)
                        pending = None
                    pt = pB.tile([128, 512], BF16, tag="pt", bufs=3)
                    nc.scalar.activation(pt[:], pss[:], ACTF.Exp)
                    if jt == 0:
                        nc.vector.tensor_copy(den[:], pt[:])
                    else:
                        nc.vector.tensor_add(den[:], den[:].bitcast(F32), pt[:])
                    for cc in range(NCH):
                        nc.tensor.matmul(
                            pso[cc][:],
                            vt[:, jt, 128 * cc : 128 * (cc + 1)],
                            pt[:],
                            start=(jt == 0),
                            stop=(jt == NJT - 1),
                        )
                # partition-sum of den, broadcast of 1/den
                psd = psB.tile([1, 512], F32, tag="f", bufs=2)
                nc.tensor.matmul(psd[:], ones128[:], den[:], start=True, stop=True)
                dtot = pB.tile([1, 512], F32R, tag="dtot", bufs=2)
                nc.vector.tensor_copy(dtot[:], psd[:])
                psb = psB.tile([128, 512], F32, tag="f", bufs=2)
                nc.tensor.matmul(psb[:], ones1[:], dtot[:], start=True, stop=True)
                rbc = pB.tile([128, 512], F32, tag="rbc", bufs=2)
                nc.vector.reciprocal(rbc[:], psb[:])
                # drain unnormalized O^T to SBUF (frees the pso banks);
                # split across ACT and DVE so neither queue stalls the next
                # q-block's exp/den chain
                osb = pB.tile([128, NCH, 512], BF16, tag="osb", bufs=2)
                for cc in range(NCH):
                    if cc % 2 == 0:
                        nc.scalar.activation(osb[:, cc, :], pso[cc][:], ACTF.Identity)
                    else:
                        nc.vector.tensor_copy(osb[:, cc, :], pso[cc][:])
                pending = (qb, osb, rbc, xb)
            if pending is not None:
                emit_epilogue(pending)
                pending = None
    return nc


# ---------------------------------------------------------------------------
# Walrus in this container rejects instructions carrying more than ~2
# sync-wait commands ("Too many sync wait commands").  Hoist excess on_wait
# entries onto nofuse NOPs placed immediately before the instruction on the
# same engine (engines issue in-order, so blocking on the NOP first is
# equivalent).
def split_sync_waits(nc, max_waits=1):
    n_split = 0
    for bb in nc.main_func.blocks:
        insts = bb.instructions
        out = []
        for inst in insts:
            si = inst.sync_info
            if si is not None and si.on_wait is not None and len(si.on_wait) > max_waits:
                waits = list(si.on_wait)
                keep = waits[-max_waits:]
                extra = waits[:-max_waits]
                for i in range(0, len(extra), max_waits):
                    chunk = extra[i : i + max_waits]
                    nop = mybir.InstNoOp(
                        name=f"{inst.name}-sw{i}",
                        sync_info=mybir.SyncInfo(on_wait=chunk, on_update=[]),
                        bass_nofuse=True,
                        engine=inst.engine,
                    )
                    out.append(nop)
                    n_split += 1
                inst.sync_info = mybir.SyncInfo(
                    on_wait=keep, on_update=list(si.on_update or [])
                )
            out.append(inst)
        bb.instructions = out
    return n_split


B, H, W = 8, 64, 64
HW = H * W
N_CORES = 8
_CACHE = {}


def _get_nc():
    if "nc" not in _CACHE:
        nc = bass.Bass()
        build(nc, HW=HW)
        split_sync_waits(nc)
        _CACHE["nc"] = nc
    return _CACHE["nc"]


def _in_maps(inputs):
    import numpy as np
    arrs = {k: np.ascontiguousarray(np.asarray(v, dtype=np.float32)) for k, v in inputs.items()}
    x = arrs.pop("x").reshape(B, C, HW)
    return [{"x": x[i], **arrs} for i in range(N_CORES)]


def kernel(**inputs):
    import numpy as np
    from concourse.bass_utils import run_bass_kernel_spmd

    nc = _get_nc()
    res = run_bass_kernel_spmd(nc, _in_maps(inputs), list(range(N_CORES)))
    out = np.stack([res.results[i]["out"] for i in range(N_CORES)])
    return out.reshape(B, C, H, W).astype(np.float32)


def kernel_traced(**inputs):
    """Like kernel() but with NTFF profiling; returns (output, BassKernelResults)."""
    import numpy as np
    from concourse.bass_utils import run_bass_kernel_spmd

    nc = _get_nc()
    res = run_bass_kernel_spmd(
        nc, _in_maps(inputs), list(range(N_CORES)), trace=True
    )
    out = np.stack([res.results[i]["out"] for i in range(N_CORES)])
    return out.reshape(B, C, H, W).astype(np.float32), res


# revision 25
# speedup vs baseline: 1.7495x; 1.3450x over previous
import sys

for _p in ("/opt/trn_rl_repo",):
    if _p not in sys.path:
        sys.path.append(_p)

"""AttnBlock (GroupNorm + single-head self-attention + residual) Bass/Tile
kernel for one NeuronCore (one batch sample), channel-major layout.

Per-core problem:  x [C=512, HW] f32
  hn = groupnorm(x, 32 groups, eps=1e-5) * gn_w + gn_b
  q/k/v = 1x1 conv (C x C) on tokens;  scores = (q k^T) / sqrt(C)
  attn = softmax(scores);  o = attn @ v;  out = x + (o @ wo^T + bo)

Layout strategy (all big matmuls in bf16: same 1 col/cycle PE rate as
f32r but enables fast-weight-load so LDWEIGHTS hides behind streaming;
bf16 also halves SBUF so Qt/Kt/V and x all stay resident -> single-pass
phase B with zero DRAM spills):
  - x kept in SBUF as bf16 after the stats pass (residual + pass-2 reads)
  - hn, Qt, Kt channel-major [c, hw];  V token-major [hw, c]
  - scores computed transposed St[j, q] = sum_c Kt[c,j] Qt[c,q]
  - exp via ACT, no max subtraction (scores ~N(0,1) by construction)
  - softmax denominator: elementwise accumulate exp tiles on DVE, then a
    ones-vector matmul for the partition sum; normalization applied to
    O^T after the PV accumulation (rank-1 ones matmul broadcasts 1/d)
  - PV: O^T[c, q] += V[j, :]^T P^T[j, q] accumulated in PSUM over all
    32 j-tiles (single pass)
  - final wo projection + bias + residual of q-block N deferred into
    q-block N+1's PE stream to fill the scores->exp->PV latency bubble
"""

from contextlib import ExitStack

import concourse.bass as bass
import concourse.tile as tile
from concourse import mybir
from concourse.masks import make_identity

F32 = mybir.dt.float32
F32R = mybir.dt.float32r
BF16 = mybir.dt.bfloat16
FP8 = mybir.dt.float8e4
DR = mybir.MatmulPerfMode.DoubleRow
AX = mybir.AxisListType
OP = mybir.AluOpType
ACTF = mybir.ActivationFunctionType
# softmax exp shift: cancels in normalization, keeps fp8 exp values in the
# healthy e4m3 range (robust to denormal flush either way; see sims)
EXP_BIAS = -2.0

C = 512
NCH = 4  # channel chunks of 128
GPC = 8  # groups per 128-channel chunk (16 channels per group)
EPS = 1e-5


def build(nc: bass.Bass, HW: int = 4096):
    SCALE_Q = float(C) ** (-0.5)
    NJB = HW // 512      # 512-col j/q blocks
    NQB = HW // 512
    NJT = HW // 128      # 128-col j tiles
    GN_N = 16 * HW       # elements per group

    x = nc.dram_tensor("x", [C, HW], F32, kind="ExternalInput")
    gn_w = nc.dram_tensor("gn_w", [C], F32, kind="ExternalInput")
    gn_b = nc.dram_tensor("gn_b", [C], F32, kind="ExternalInput")
    wq = nc.dram_tensor("wq", [C, C], F32, kind="ExternalInput")
    bq = nc.dram_tensor("bq", [C], F32, kind="ExternalInput")
    wk = nc.dram_tensor("wk", [C, C], F32, kind="ExternalInput")
    bk = nc.dram_tensor("bk", [C], F32, kind="ExternalInput")
    wv = nc.dram_tensor("wv", [C, C], F32, kind="ExternalInput")
    bv = nc.dram_tensor("bv", [C], F32, kind="ExternalInput")
    wo = nc.dram_tensor("wo", [C, C], F32, kind="ExternalInput")
    bo = nc.dram_tensor("bo", [C], F32, kind="ExternalInput")
    out = nc.dram_tensor("out", [C, HW], F32, kind="ExternalOutput")

    x_r = x.rearrange("(c p) q -> p c q", p=128)
    out_r = out.rearrange("(c p) q -> p c q", p=128)

    with tile.TileContext(nc) as tc, ExitStack() as ctx:
        pconst = ctx.enter_context(tc.tile_pool(name="const", bufs=1))
        ppersist = ctx.enter_context(tc.tile_pool(name="persist", bufs=1))

        # ---- constants ----
        identity = pconst.tile([128, 128], F32, tag="ident")
        make_identity(nc, identity[:])
        ones128_f = pconst.tile([128, 1], F32, tag="ones128_f")
        nc.gpsimd.memset(ones128_f[:], 1.0)
        ones128 = pconst.tile([128, 1], F32R, tag="ones128")
        nc.vector.tensor_copy(ones128[:], ones128_f[:])
        ones1_f = pconst.tile([1, 128], F32, tag="ones1_f")
        nc.gpsimd.memset(ones1_f[:], 1.0)
        ones1 = pconst.tile([1, 128], F32R, tag="ones1")
        nc.vector.tensor_copy(ones1[:], ones1_f[:])
        # group indicator matrices: ind8[c, g] = e8[g, c] = (c // 16 == g)
        ind8_f = pconst.tile([128, GPC], F32, tag="ind8_f")
        nc.gpsimd.memset(ind8_f[:], 1.0)
        nc.gpsimd.affine_select(
            out=ind8_f[:], in_=ind8_f[:], compare_op=OP.is_ge, fill=0.0,
            base=0, channel_multiplier=1, pattern=[[-16, GPC]],
        )
        nc.gpsimd.affine_select(
            out=ind8_f[:], in_=ind8_f[:], compare_op=OP.is_ge, fill=0.0,
            base=15, channel_multiplier=-1, pattern=[[16, GPC]],
        )
        ind8 = pconst.tile([128, GPC], F32R, tag="ind8")
        nc.vector.tensor_copy(ind8[:], ind8_f[:])
        e8_f = pconst.tile([GPC, 128], F32, tag="e8_f")
        nc.gpsimd.memset(e8_f[:], 1.0)
        nc.gpsimd.affine_select(
            out=e8_f[:], in_=e8_f[:], compare_op=OP.is_ge, fill=0.0,
            base=0, channel_multiplier=-16, pattern=[[1, 128]],
        )
        nc.gpsimd.affine_select(
            out=e8_f[:], in_=e8_f[:], compare_op=OP.is_ge, fill=0.0,
            base=15, channel_multiplier=16, pattern=[[-1, 128]],
        )
        e8 = pconst.tile([GPC, 128], F32R, tag="e8")
        nc.vector.tensor_copy(e8[:], e8_f[:])

        gnw4 = pconst.tile([128, NCH], F32, tag="gnw4")
        gnb4 = pconst.tile([128, NCH], F32, tag="gnb4")
        bq4 = pconst.tile([128, NCH], F32, tag="bq4")
        bk4 = pconst.tile([128, NCH], F32, tag="bk4")
        bo4 = pconst.tile([128, NCH], F32, tag="bo4")
        for t, src in ((gnw4, gn_w), (gnb4, gn_b), (bq4, bq), (bk4, bk), (bo4, bo)):
            nc.sync.dma_start(out=t[:], in_=src.rearrange("(c p) -> p c", p=128))
        bv_row = pconst.tile([1, C], F32, tag="bv_row")
        nc.sync.dma_start(out=bv_row[:], in_=bv.rearrange("(a i) -> a i", a=1))
        bv_row_r = pconst.tile([1, C], F32R, tag="bv_row_r")
        nc.vector.tensor_copy(bv_row_r[:], bv_row[:])
        bv_bcast = pconst.tile([128, C], F32, tag="bv_bcast")

        eps_t = pconst.tile([GPC, 1], F32, tag="eps_t")
        nc.gpsimd.memset(eps_t[:], EPS)
        expb_t = pconst.tile([128, 1], F32, tag="expb_t")
        nc.gpsimd.memset(expb_t[:], EXP_BIAS)
        bnst = pconst.tile([128, NCH, NJB, 6], F32, tag="bnst")
        cmv = pconst.tile([128, NCH, 2], F32, tag="cmv")
        ch_stats_r = pconst.tile([128, NCH, 2], F32R, tag="ch_stats_r")
        scale4 = pconst.tile([128, NCH], F32, tag="scale4")
        shift4 = pconst.tile([128, NCH], F32, tag="shift4")

        # ---- persistent tensors (all resident, no spills) ----
        x_sb = ppersist.tile([128, NCH, HW], BF16, tag="x_sb")
        qt = ppersist.tile([128, NCH, HW], FP8, tag="qt")
        kt = ppersist.tile([128, NCH, HW], FP8, tag="kt")
        vt = ppersist.tile([128, NJT, C], FP8, tag="vt")
        woT = ppersist.tile([128, NCH, C], BF16, tag="woT")

        # ---- phase A ----
        with tc.tile_pool(name="wqkv", bufs=1) as pwqkv, \
             tc.tile_pool(name="stream", bufs=2) as pstream:
            wqT = pwqkv.tile([128, NCH, C], BF16, tag="wqT")
            wkT = pwqkv.tile([128, NCH, C], BF16, tag="wkT")
            wvT = pwqkv.tile([128, NCH, C], BF16, tag="wvT")

            with tc.tile_pool(name="psA", bufs=1, space="PSUM") as psA:
                with tc.tile_pool(name="scrA", bufs=2) as pscr, \
                     tc.tile_pool(name="raw", bufs=2) as praw:
                    # weight DMAs issued first so transposes can fill the
                    # stats pass on the PE
                    raws = []
                    for w_ext in (wq, wk, wv, wo):
                        raw = praw.tile([128, NCH, C], F32, tag="raw")
                        w_r = w_ext.rearrange("(c p) i -> p c i", p=128)
                        for co in range(NCH):
                            # chunked so the transfers spread across DMA queues
                            nc.sync.dma_start(
                                out=raw[:, co, :], in_=w_r[:, co, :]
                            )
                        raws.append(raw)
                    # ---- pass 1: GN statistics + bf16 copy of x into SBUF ----
                    # all elementwise work on DVE (bn_stats one-pass mean/var)
                    # so the ACT queue stays free for the transpose PSUM drains
                    for jb in range(NJB):
                        x_in = pstream.tile([128, NCH, 512], F32, tag="xin")
                        nc.sync.dma_start(
                            out=x_in[:], in_=x_r[:, :, 512 * jb : 512 * (jb + 1)]
                        )
                        for ci in range(NCH):
                            nc.vector.tensor_copy(
                                x_sb[:, ci, 512 * jb : 512 * (jb + 1)],
                                x_in[:, ci, :],
                            )
                            nc.vector.bn_stats(
                                bnst[:, ci, jb, :], x_in[:, ci, :]
                            )
                    # weight transposes: wT[:, ci, co*128:..] = W[co blk, ci blk].T
                    if True:
                        for raw, wT in zip(raws, (wqT, wkT, wvT, woT)):
                            for co in range(NCH):
                                for ci in range(NCH):
                                    ps = psA.tile([128, 128], F32, tag="m", bufs=4)
                                    nc.tensor.transpose(
                                        ps[:],
                                        raw[:, co, 128 * ci : 128 * (ci + 1)],
                                        identity[:],
                                    )
                                    nc.scalar.activation(
                                        wT[:, ci, 128 * co : 128 * (co + 1)],
                                        ps[:],
                                        ACTF.Identity,
                                    )
                        # bv broadcast tile (rank-1 matmul)
                        psbv = psA.tile([128, C], F32, tag="m", bufs=4)
                        nc.tensor.matmul(
                            psbv[:], ones1[:], bv_row_r[:], start=True, stop=True
                        )
                        nc.scalar.activation(bv_bcast[:], psbv[:], ACTF.Identity)
                    # combine stats -> per-channel scale/shift (batched over ci
                    # to minimize cross-engine dependency hops)
                    for ci in range(NCH):
                        nc.vector.bn_aggr(cmv[:, ci, :], bnst[:, ci, :, :])
                    m2 = pscr.tile([128, NCH, 1], F32, tag="st_m2")
                    nc.vector.tensor_mul(m2[:], cmv[:, :, 0:1], cmv[:, :, 0:1])
                    nc.vector.tensor_copy(ch_stats_r[:, :, 0:1], cmv[:, :, 0:1])
                    nc.vector.tensor_add(
                        ch_stats_r[:, :, 1:2], cmv[:, :, 1:2], m2[:]
                    )
                    psg = psA.tile([GPC, NCH, 2], F32, tag="t", bufs=2)
                    nc.tensor.matmul(
                        psg[:], ind8[:], ch_stats_r[:], start=True, stop=True
                    )
                    meang = pscr.tile([GPC, NCH, 1], F32, tag="st_mean")
                    ex2g = pscr.tile([GPC, NCH, 1], F32, tag="st_ex2")
                    nc.vector.tensor_scalar_mul(meang[:], psg[:, :, 0:1], 1.0 / 16.0)
                    nc.vector.tensor_scalar_mul(ex2g[:], psg[:, :, 1:2], 1.0 / 16.0)
                    varg = pscr.tile([GPC, NCH, 1], F32, tag="st_var")
                    nc.vector.tensor_mul(varg[:], meang[:], meang[:])
                    nc.vector.tensor_sub(varg[:], ex2g[:], varg[:])
                    stdg = pscr.tile([GPC, NCH, 1], F32, tag="st_std")
                    nc.scalar.activation(stdg[:], varg[:], ACTF.Sqrt, bias=eps_t[:])
                    rstdg = pscr.tile([GPC, NCH, 1], F32, tag="st_rstd")
                    nc.vector.reciprocal(rstdg[:], stdg[:])
                    st2 = pscr.tile([GPC, NCH, 2], F32R, tag="st2")
                    nc.vector.tensor_copy(st2[:, :, 0:1], rstdg[:])
                    nc.vector.tensor_copy(st2[:, :, 1:2], meang[:])
                    pse = psA.tile([128, NCH, 2], F32, tag="t", bufs=2)
                    nc.tensor.matmul(pse[:], e8[:], st2[:], start=True, stop=True)
                    # scale = rstd * gamma ; shift = beta - mean * scale
                    nc.vector.tensor_mul(scale4[:], pse[:, :, 0:1], gnw4[:])
                    tmp4 = pscr.tile([128, NCH], F32, tag="st_tmp")
                    nc.vector.tensor_mul(tmp4[:], pse[:, :, 1:2], scale4[:])
                    nc.vector.tensor_sub(shift4[:], gnb4[:], tmp4[:])

                    # ---- pass 2: GN apply + Q/K/V projections (from SBUF x) ----
                    for jb in range(NJB):
                        hn = pscr.tile([128, NCH, 512], BF16, tag="hn")
                        for ci in range(NCH):
                            nc.scalar.activation(
                                hn[:, ci, :],
                                x_sb[:, ci, 512 * jb : 512 * (jb + 1)],
                                ACTF.Identity,
                                scale=scale4[:, ci : ci + 1],
                                bias=shift4[:, ci : ci + 1],
                            )
                        # Q and K, channel-major, fp8 (1/sqrt(C) folded into
                        # the exp scale in phase B, so q/k stay ~N(0,1))
                        for wT, dst, bias_ap in (
                            (wqT, qt, bq4),
                            (wkT, kt, bk4),
                        ):
                            for co in range(NCH):
                                psq = psA.tile([128, 512], F32, tag="m", bufs=4)
                                for ci in range(NCH):
                                    nc.tensor.matmul(
                                        psq[:],
                                        wT[:, ci, 128 * co : 128 * (co + 1)],
                                        hn[:, ci, :],
                                        start=(ci == 0),
                                        stop=(ci == NCH - 1),
                                    )
                                nc.scalar.activation(
                                    dst[:, co, 512 * jb : 512 * (jb + 1)],
                                    psq[:],
                                    ACTF.Identity,
                                    bias=bias_ap[:, co : co + 1],
                                )
                        # V[j, c] per j-subtile, token-major
                        for jtl in range(4):
                            psv = psA.tile([128, 512], F32, tag="m", bufs=4)
                            for ci in range(NCH):
                                nc.tensor.matmul(
                                    psv[:],
                                    hn[:, ci, 128 * jtl : 128 * (jtl + 1)],
                                    wvT[:, ci, :],
                                    start=(ci == 0),
                                    stop=(ci == NCH - 1),
                                )
                            nc.vector.tensor_add(
                                vt[:, 4 * jb + jtl, :], psv[:], bv_bcast[:]
                            )

        # ---- phase B: single pass over all q-blocks, full K/V resident ----
        with (
            tc.tile_pool(name="poolB", bufs=1) as pB,
            tc.tile_pool(name="psB", bufs=1, space="PSUM") as psB,
        ):
            pending = None

            def emit_epilogue(p):
                # deferred final projection + bias + residual for a prior
                # q-block; spliced into the next q-block's PE stream so it
                # fills the scores->exp->PV latency bubble
                e_qb, e_osb, e_rbc, e_xb = p
                outs = pB.tile([128, NCH, 512], F32, tag="outs", bufs=2)
                for co in range(NCH):
                    psf = psB.tile([128, 512], F32, tag="f", bufs=2)
                    for cc in range(NCH):
                        nc.tensor.matmul(
                            psf[:],
                            woT[:, cc, 128 * co : 128 * (co + 1)],
                            e_osb[:, cc, :],
                            start=(cc == 0),
                            stop=(cc == NCH - 1),
                        )
                    nc.vector.tensor_mul(outs[:, co, :], psf[:], e_rbc[:])
                    nc.vector.tensor_add(
                        outs[:, co, :], outs[:, co, :], e_xb[:, co, :]
                    )
                nc.sync.dma_start(
                    out=out_r[:, :, 512 * e_qb : 512 * (e_qb + 1)], in_=outs[:]
                )

            for qb in range(NQB):
                # residual + output bias staged early on DVE (off the ACT
                # queue, consumed by the deferred epilogue next q-block)
                xb = pB.tile([128, NCH, 512], BF16, tag="xb", bufs=2)
                for co in range(NCH):
                    nc.vector.tensor_scalar_add(
                        xb[:, co, :],
                        x_sb[:, co, 512 * qb : 512 * (qb + 1)],
                        bo4[:, co : co + 1],
                    )
                # den accumulated in f32 (DVE converts bf16 exp tiles on read);
                # stored as f32r so the ones-matmul can consume it directly
                den = pB.tile([128, 512], F32R, tag="den", bufs=2)
                pso = [
                    psB.tile([128, 512], F32, tag="o", bufs=4, name="pso")
                    for _ in range(NCH)
                ]
                for jt2 in range(NJT // 2):
                    # two score tiles -> exp -> one DoubleRow PV round
                    pt8 = pB.tile([128, 2, 512], FP8, tag="pt", bufs=3)
                    for sub in range(2):
                        jt = 2 * jt2 + sub
                        pss = psB.tile([128, 512], F32, tag="s", bufs=2)
                        for ph in range(2):
                            nc.tensor.matmul(
                                pss[:],
                                kt[:, 2 * ph : 2 * ph + 2, 128 * jt : 128 * (jt + 1)],
                                qt[:, 2 * ph : 2 * ph + 2, 512 * qb : 512 * (qb + 1)],
                                start=(ph == 0),
                                stop=(ph == 1),
                                perf_mode=DR,
                            )
                        if jt == 0 and pending is not None:
                            emit_epilogue(pending)
                            pending = None
                        nc.scalar.activation(
                            pt8[:, sub, :], pss[:], ACTF.Exp,
                            scale=SCALE_Q, bias=expb_t[:],
                        )
                        if jt == 0:
                            nc.vector.tensor_copy(den[:], pt8[:, sub, :])
                        else:
                            nc.vector.tensor_add(
                                den[:], den[:].bitcast(F32), pt8[:, sub, :]
                            )
                    for cc in range(NCH):
                        nc.tensor.matmul(
                            pso[cc][:],
                            vt[:, 2 * jt2 : 2 * jt2 + 2, 128 * cc : 128 * (cc + 1)],
                            pt8[:],
                            start=(jt2 == 0),
                            stop=(jt2 == NJT // 2 - 1),
                            perf_mode=DR,
                        )
                # partition-sum of den, broadcast of 1/den
                psd = psB.tile([1, 512], F32, tag="f", bufs=2)
                nc.tensor.matmul(psd[:], ones128[:], den[:], start=True, stop=True)
                dtot = pB.tile([1, 512], F32R, tag="dtot", bufs=2)
                nc.vector.tensor_copy(dtot[:], psd[:])
                psb = psB.tile([128, 512], F32, tag="f", bufs=2)
                nc.tensor.matmul(psb[:], ones1[:], dtot[:], start=True, stop=True)
                rbc = pB.tile([128, 512], F32, tag="rbc", bufs=2)
                nc.vector.reciprocal(rbc[:], psb[:])
                # drain unnormalized O^T to SBUF (frees the pso banks);
                # split across ACT and DVE so neither queue stalls the next
                # q-block's exp/den chain
                osb = pB.tile([128, NCH, 512], BF16, tag="osb", bufs=2)
                for cc in range(NCH):
                    if cc % 2 == 0:
                        nc.scalar.activation(osb[:, cc, :], pso[cc][:], ACTF.Identity)
                    else:
                        nc.vector.tensor_copy(osb[:, cc, :], pso[cc][:])
                pending = (qb, osb, rbc, xb)
            if pending is not None:
                emit_epilogue(pending)
                pending = None
    return nc


# ---------------------------------------------------------------------------
# Walrus in this container rejects instructions carrying more than ~2
# sync-wait commands ("Too many sync wait commands").  Hoist excess on_wait
# entries onto nofuse NOPs placed immediately before the instruction on the
# same engine (engines issue in-order, so blocking on the NOP first is
# equivalent).
def split_sync_waits(nc, max_waits=1):
    n_split = 0
    for bb in nc.main_func.blocks:
        insts = bb.instructions
        out = []
        for inst in insts:
            si = inst.sync_info
            if si is not None and si.on_wait is not None and len(si.on_wait) > max_waits:
                waits = list(si.on_wait)
                keep = waits[-max_waits:]
                extra = waits[:-max_waits]
                for i in range(0, len(extra), max_waits):
                    chunk = extra[i : i + max_waits]
                    nop = mybir.InstNoOp(
                        name=f"{inst.name}-sw{i}",
                        sync_info=mybir.SyncInfo(on_wait=chunk, on_update=[]),
                        bass_nofuse=True,
                        engine=inst.engine,
                    )
                    out.append(nop)
                    n_split += 1
                inst.sync_info = mybir.SyncInfo(
                    on_wait=keep, on_update=list(si.on_update or [])
                )
            out.append(inst)
        bb.instructions = out
    return n_split


B, H, W = 8, 64, 64
HW = H * W
N_CORES = 8
_CACHE = {}


def _get_nc():
    if "nc" not in _CACHE:
        nc = bass.Bass()
        build(nc, HW=HW)
        split_sync_waits(nc)
        _CACHE["nc"] = nc
    return _CACHE["nc"]


def _in_maps(inputs):
    import numpy as np
    arrs = {k: np.ascontiguousarray(np.asarray(v, dtype=np.float32)) for k, v in inputs.items()}
    x = arrs.pop("x").reshape(B, C, HW)
    return [{"x": x[i], **arrs} for i in range(N_CORES)]


def kernel(**inputs):
    import numpy as np
    from concourse.bass_utils import run_bass_kernel_spmd

    nc = _get_nc()
    res = run_bass_kernel_spmd(nc, _in_maps(inputs), list(range(N_CORES)))
    out = np.stack([res.results[i]["out"] for i in range(N_CORES)])
    return out.reshape(B, C, H, W).astype(np.float32)


def kernel_traced(**inputs):
    """Like kernel() but with NTFF profiling; returns (output, BassKernelResults)."""
    import numpy as np
    from concourse.bass_utils import run_bass_kernel_spmd

    nc = _get_nc()
    res = run_bass_kernel_spmd(
        nc, _in_maps(inputs), list(range(N_CORES)), trace=True
    )
    out = np.stack([res.results[i]["out"] for i in range(N_CORES)])
    return out.reshape(B, C, H, W).astype(np.float32), res


# revision 33
# speedup vs baseline: 1.9513x; 1.1154x over previous
import sys

for _p in ("/opt/trn_rl_repo",):
    if _p not in sys.path:
        sys.path.append(_p)

"""AttnBlock (GroupNorm + single-head self-attention + residual) Bass/Tile
kernel for one NeuronCore (one batch sample), channel-major layout.

Per-core problem:  x [C=512, HW] f32
  hn = groupnorm(x, 32 groups, eps=1e-5) * gn_w + gn_b
  q/k/v = 1x1 conv (C x C) on tokens;  scores = (q k^T) / sqrt(C)
  attn = softmax(scores);  o = attn @ v;  out = x + (o @ wo^T + bo)

Layout strategy (all big matmuls in bf16: same 1 col/cycle PE rate as
f32r but enables fast-weight-load so LDWEIGHTS hides behind streaming;
bf16 also halves SBUF so Qt/Kt/V and x all stay resident -> single-pass
phase B with zero DRAM spills):
  - x kept in SBUF as bf16 after the stats pass (residual + pass-2 reads)
  - hn, Qt, Kt channel-major [c, hw];  V token-major [hw, c]
  - scores computed transposed St[j, q] = sum_c Kt[c,j] Qt[c,q]
  - exp via ACT, no max subtraction (scores ~N(0,1) by construction)
  - softmax denominator: elementwise accumulate exp tiles on DVE, then a
    ones-vector matmul for the partition sum; normalization applied to
    O^T after the PV accumulation (rank-1 ones matmul broadcasts 1/d)
  - PV: O^T[c, q] += V[j, :]^T P^T[j, q] accumulated in PSUM over all
    32 j-tiles (single pass)
  - final wo projection + bias + residual of q-block N deferred into
    q-block N+1's PE stream to fill the scores->exp->PV latency bubble
"""

from contextlib import ExitStack

import concourse.bass as bass
import concourse.tile as tile
from concourse import mybir
from concourse.masks import make_identity

F32 = mybir.dt.float32
F32R = mybir.dt.float32r
BF16 = mybir.dt.bfloat16
FP8 = mybir.dt.float8e4
DR = mybir.MatmulPerfMode.DoubleRow
AX = mybir.AxisListType
OP = mybir.AluOpType
ACTF = mybir.ActivationFunctionType
# softmax exp shift: cancels in normalization, keeps fp8 exp values in the
# healthy e4m3 range (robust to denormal flush either way; see sims)
EXP_BIAS = -2.0

C = 512
NCH = 4  # channel chunks of 128
GPC = 8  # groups per 128-channel chunk (16 channels per group)
EPS = 1e-5


def build(nc: bass.Bass, HW: int = 4096):
    SCALE_Q = float(C) ** (-0.5)
    NJB = HW // 512      # 512-col j/q blocks
    NQB = HW // 512
    NJT = HW // 128      # 128-col j tiles
    GN_N = 16 * HW       # elements per group

    x = nc.dram_tensor("x", [C, HW], F32, kind="ExternalInput")
    gn_w = nc.dram_tensor("gn_w", [C], F32, kind="ExternalInput")
    gn_b = nc.dram_tensor("gn_b", [C], F32, kind="ExternalInput")
    wq = nc.dram_tensor("wq", [C, C], F32, kind="ExternalInput")
    bq = nc.dram_tensor("bq", [C], F32, kind="ExternalInput")
    wk = nc.dram_tensor("wk", [C, C], F32, kind="ExternalInput")
    bk = nc.dram_tensor("bk", [C], F32, kind="ExternalInput")
    wv = nc.dram_tensor("wv", [C, C], F32, kind="ExternalInput")
    bv = nc.dram_tensor("bv", [C], F32, kind="ExternalInput")
    wo = nc.dram_tensor("wo", [C, C], F32, kind="ExternalInput")
    bo = nc.dram_tensor("bo", [C], F32, kind="ExternalInput")
    out = nc.dram_tensor("out", [C, HW], F32, kind="ExternalOutput")

    x_r = x.rearrange("(c p) q -> p c q", p=128)
    out_r = out.rearrange("(c p) q -> p c q", p=128)

    with tile.TileContext(nc) as tc, ExitStack() as ctx:
        pconst = ctx.enter_context(tc.tile_pool(name="const", bufs=1))
        ppersist = ctx.enter_context(tc.tile_pool(name="persist", bufs=1))

        # ---- constants ----
        identity = pconst.tile([128, 128], F32, tag="ident")
        make_identity(nc, identity[:])
        ones128_f = pconst.tile([128, 1], F32, tag="ones128_f")
        nc.gpsimd.memset(ones128_f[:], 1.0)
        ones128 = pconst.tile([128, 1], F32R, tag="ones128")
        nc.vector.tensor_copy(ones128[:], ones128_f[:])
        ones1_f = pconst.tile([1, 128], F32, tag="ones1_f")
        nc.gpsimd.memset(ones1_f[:], 1.0)
        ones1 = pconst.tile([1, 128], F32R, tag="ones1")
        nc.vector.tensor_copy(ones1[:], ones1_f[:])
        # group indicator matrices: ind8[c, g] = e8[g, c] = (c // 16 == g)
        ind8_f = pconst.tile([128, GPC], F32, tag="ind8_f")
        nc.gpsimd.memset(ind8_f[:], 1.0)
        nc.gpsimd.affine_select(
            out=ind8_f[:], in_=ind8_f[:], compare_op=OP.is_ge, fill=0.0,
            base=0, channel_multiplier=1, pattern=[[-16, GPC]],
        )
        nc.gpsimd.affine_select(
            out=ind8_f[:], in_=ind8_f[:], compare_op=OP.is_ge, fill=0.0,
            base=15, channel_multiplier=-1, pattern=[[16, GPC]],
        )
        ind8 = pconst.tile([128, GPC], F32R, tag="ind8")
        nc.vector.tensor_copy(ind8[:], ind8_f[:])
        e8_f = pconst.tile([GPC, 128], F32, tag="e8_f")
        nc.gpsimd.memset(e8_f[:], 1.0)
        nc.gpsimd.affine_select(
            out=e8_f[:], in_=e8_f[:], compare_op=OP.is_ge, fill=0.0,
            base=0, channel_multiplier=-16, pattern=[[1, 128]],
        )
        nc.gpsimd.affine_select(
            out=e8_f[:], in_=e8_f[:], compare_op=OP.is_ge, fill=0.0,
            base=15, channel_multiplier=16, pattern=[[-1, 128]],
        )
        e8 = pconst.tile([GPC, 128], F32R, tag="e8")
        nc.vector.tensor_copy(e8[:], e8_f[:])

        gnw4 = pconst.tile([128, NCH], F32, tag="gnw4")
        gnb4 = pconst.tile([128, NCH], F32, tag="gnb4")
        bq4 = pconst.tile([128, NCH], F32, tag="bq4")
        bk4 = pconst.tile([128, NCH], F32, tag="bk4")
        bo4 = pconst.tile([128, NCH], F32, tag="bo4")
        for t, src in ((gnw4, gn_w), (gnb4, gn_b), (bq4, bq), (bk4, bk), (bo4, bo)):
            nc.sync.dma_start(out=t[:], in_=src.rearrange("(c p) -> p c", p=128))
        bv_row = pconst.tile([1, C], F32, tag="bv_row")
        nc.sync.dma_start(out=bv_row[:], in_=bv.rearrange("(a i) -> a i", a=1))
        bv_row_r = pconst.tile([1, C], F32R, tag="bv_row_r")
        nc.vector.tensor_copy(bv_row_r[:], bv_row[:])
        bv_bcast = pconst.tile([128, C], F32, tag="bv_bcast")

        eps_t = pconst.tile([GPC, 1], F32, tag="eps_t")
        nc.gpsimd.memset(eps_t[:], EPS)
        expb_t = pconst.tile([128, 1], F32, tag="expb_t")
        nc.gpsimd.memset(expb_t[:], EXP_BIAS)
        bnst = pconst.tile([128, NCH, NJB, 6], F32, tag="bnst")
        cmv = pconst.tile([128, NCH, 2], F32, tag="cmv")
        ch_stats_r = pconst.tile([128, NCH, 2], F32R, tag="ch_stats_r")
        scale4 = pconst.tile([128, NCH], F32, tag="scale4")
        shift4 = pconst.tile([128, NCH], F32, tag="shift4")

        # ---- persistent tensors (all resident, no spills) ----
        x_sb = ppersist.tile([128, NCH, HW], BF16, tag="x_sb")
        qt = ppersist.tile([128, NCH, HW], FP8, tag="qt")
        kt = ppersist.tile([128, NCH, HW], FP8, tag="kt")
        vt = ppersist.tile([128, NJT, C], FP8, tag="vt")
        woT = ppersist.tile([128, NCH, C], BF16, tag="woT")

        # ---- phase A ----
        with tc.tile_pool(name="wqkv", bufs=1) as pwqkv, \
             tc.tile_pool(name="stream", bufs=2) as pstream:
            wqT = pwqkv.tile([128, NCH, C], BF16, tag="wqT")
            wkT = pwqkv.tile([128, NCH, C], BF16, tag="wkT")
            wvT = pwqkv.tile([128, NCH, C], BF16, tag="wvT")

            with tc.tile_pool(name="psA", bufs=1, space="PSUM") as psA:
                with tc.tile_pool(name="scrA", bufs=2) as pscr, \
                     tc.tile_pool(name="raw", bufs=2) as praw:
                    # weight DMAs issued first so transposes can fill the
                    # stats pass on the PE
                    raws = []
                    for w_ext in (wq, wk, wv, wo):
                        raw = praw.tile([128, NCH, C], F32, tag="raw")
                        w_r = w_ext.rearrange("(c p) i -> p c i", p=128)
                        for co in range(NCH):
                            # chunked so the transfers spread across DMA queues
                            nc.sync.dma_start(
                                out=raw[:, co, :], in_=w_r[:, co, :]
                            )
                        raws.append(raw)
                    # ---- pass 1: GN statistics + bf16 copy of x into SBUF ----
                    # all elementwise work on DVE (bn_stats one-pass mean/var)
                    # so the ACT queue stays free for the transpose PSUM drains
                    for jb in range(NJB):
                        x_in = pstream.tile([128, NCH, 512], F32, tag="xin")
                        nc.sync.dma_start(
                            out=x_in[:], in_=x_r[:, :, 512 * jb : 512 * (jb + 1)]
                        )
                        # bf16 copy on the (otherwise idle) GPSIMD engine;
                        # bn_stats on DVE
                        nc.gpsimd.tensor_copy(
                            x_sb[:, :, 512 * jb : 512 * (jb + 1)], x_in[:]
                        )
                        for ci in range(NCH):
                            nc.vector.bn_stats(
                                bnst[:, ci, jb, :], x_in[:, ci, :]
                            )
                    # weight transposes: wT[:, ci, co*128:..] = W[co blk, ci blk].T
                    if True:
                        for raw, wT in zip(raws, (wqT, wkT, wvT, woT)):
                            for co in range(NCH):
                                for ci in range(NCH):
                                    ps = psA.tile([128, 128], F32, tag="m", bufs=4)
                                    nc.tensor.transpose(
                                        ps[:],
                                        raw[:, co, 128 * ci : 128 * (ci + 1)],
                                        identity[:],
                                    )
                                    nc.scalar.activation(
                                        wT[:, ci, 128 * co : 128 * (co + 1)],
                                        ps[:],
                                        ACTF.Identity,
                                    )
                        # bv broadcast tile (rank-1 matmul)
                        psbv = psA.tile([128, C], F32, tag="m", bufs=4)
                        nc.tensor.matmul(
                            psbv[:], ones1[:], bv_row_r[:], start=True, stop=True
                        )
                        nc.scalar.activation(bv_bcast[:], psbv[:], ACTF.Identity)
                    # combine stats -> per-channel scale/shift (batched over ci
                    # to minimize cross-engine dependency hops)
                    for ci in range(NCH):
                        nc.vector.bn_aggr(cmv[:, ci, :], bnst[:, ci, :, :])
                    m2 = pscr.tile([128, NCH, 1], F32, tag="st_m2")
                    nc.vector.tensor_mul(m2[:], cmv[:, :, 0:1], cmv[:, :, 0:1])
                    nc.vector.tensor_copy(ch_stats_r[:, :, 0:1], cmv[:, :, 0:1])
                    nc.vector.tensor_add(
                        ch_stats_r[:, :, 1:2], cmv[:, :, 1:2], m2[:]
                    )
                    psg = psA.tile([GPC, NCH, 2], F32, tag="t", bufs=2)
                    nc.tensor.matmul(
                        psg[:], ind8[:], ch_stats_r[:], start=True, stop=True
                    )
                    meang = pscr.tile([GPC, NCH, 1], F32, tag="st_mean")
                    ex2g = pscr.tile([GPC, NCH, 1], F32, tag="st_ex2")
                    nc.vector.tensor_scalar_mul(meang[:], psg[:, :, 0:1], 1.0 / 16.0)
                    nc.vector.tensor_scalar_mul(ex2g[:], psg[:, :, 1:2], 1.0 / 16.0)
                    varg = pscr.tile([GPC, NCH, 1], F32, tag="st_var")
                    nc.vector.tensor_mul(varg[:], meang[:], meang[:])
                    nc.vector.tensor_sub(varg[:], ex2g[:], varg[:])
                    stdg = pscr.tile([GPC, NCH, 1], F32, tag="st_std")
                    nc.scalar.activation(stdg[:], varg[:], ACTF.Sqrt, bias=eps_t[:])
                    rstdg = pscr.tile([GPC, NCH, 1], F32, tag="st_rstd")
                    nc.vector.reciprocal(rstdg[:], stdg[:])
                    st2 = pscr.tile([GPC, NCH, 2], F32R, tag="st2")
                    nc.vector.tensor_copy(st2[:, :, 0:1], rstdg[:])
                    nc.vector.tensor_copy(st2[:, :, 1:2], meang[:])
                    pse = psA.tile([128, NCH, 2], F32, tag="t", bufs=2)
                    nc.tensor.matmul(pse[:], e8[:], st2[:], start=True, stop=True)
                    # scale = rstd * gamma ; shift = beta - mean * scale
                    nc.vector.tensor_mul(scale4[:], pse[:, :, 0:1], gnw4[:])
                    tmp4 = pscr.tile([128, NCH], F32, tag="st_tmp")
                    nc.vector.tensor_mul(tmp4[:], pse[:, :, 1:2], scale4[:])
                    nc.vector.tensor_sub(shift4[:], gnb4[:], tmp4[:])

                    # ---- pass 2: GN apply + Q/K/V projections (from SBUF x) ----
                    for jb in range(NJB):
                        hn = pscr.tile([128, NCH, 512], BF16, tag="hn")
                        for ci in range(NCH):
                            nc.scalar.activation(
                                hn[:, ci, :],
                                x_sb[:, ci, 512 * jb : 512 * (jb + 1)],
                                ACTF.Identity,
                                scale=scale4[:, ci : ci + 1],
                                bias=shift4[:, ci : ci + 1],
                            )
                        # Q and K, channel-major, fp8 (1/sqrt(C) folded into
                        # the exp scale in phase B, so q/k stay ~N(0,1))
                        for wT, dst, bias_ap in (
                            (wqT, qt, bq4),
                            (wkT, kt, bk4),
                        ):
                            for co in range(NCH):
                                psq = psA.tile([128, 512], F32, tag="m", bufs=4)
                                for ci in range(NCH):
                                    nc.tensor.matmul(
                                        psq[:],
                                        wT[:, ci, 128 * co : 128 * (co + 1)],
                                        hn[:, ci, :],
                                        start=(ci == 0),
                                        stop=(ci == NCH - 1),
                                    )
                                nc.scalar.activation(
                                    dst[:, co, 512 * jb : 512 * (jb + 1)],
                                    psq[:],
                                    ACTF.Identity,
                                    bias=bias_ap[:, co : co + 1],
                                )
                        # V[j, c] per j-subtile, token-major
                        for jtl in range(4):
                            psv = psA.tile([128, 512], F32, tag="m", bufs=4)
                            for ci in range(NCH):
                                nc.tensor.matmul(
                                    psv[:],
                                    hn[:, ci, 128 * jtl : 128 * (jtl + 1)],
                                    wvT[:, ci, :],
                                    start=(ci == 0),
                                    stop=(ci == NCH - 1),
                                )
                            nc.vector.tensor_add(
                                vt[:, 4 * jb + jtl, :], psv[:], bv_bcast[:]
                            )

        # ---- phase B: single pass over all q-blocks, full K/V resident ----
        with (
            tc.tile_pool(name="poolB", bufs=1) as pB,
            tc.tile_pool(name="psB", bufs=1, space="PSUM") as psB,
        ):
            pending = None

            def emit_epilogue(p):
                # deferred final projection + bias + residual for a prior
                # q-block; spliced into the next q-block's PE stream so it
                # fills the scores->exp->PV latency bubble
                e_qb, e_osb, e_rbc, e_xb = p
                outs = pB.tile([128, NCH, 512], F32, tag="outs", bufs=2)
                for co in range(NCH):
                    psf = psB.tile([128, 512], F32, tag="f", bufs=1)
                    for cc in range(NCH):
                        nc.tensor.matmul(
                            psf[:],
                            woT[:, cc, 128 * co : 128 * (co + 1)],
                            e_osb[:, cc, :],
                            start=(cc == 0),
                            stop=(cc == NCH - 1),
                        )
                    nc.vector.tensor_mul(outs[:, co, :], psf[:], e_rbc[:])
                    nc.vector.tensor_add(
                        outs[:, co, :], outs[:, co, :], e_xb[:, co, :]
                    )
                nc.sync.dma_start(
                    out=out_r[:, :, 512 * e_qb : 512 * (e_qb + 1)], in_=outs[:]
                )

            for qb in range(NQB):
                # residual + output bias staged early on DVE (off the ACT
                # queue, consumed by the deferred epilogue next q-block)
                xb = pB.tile([128, NCH, 512], BF16, tag="xb", bufs=2)
                for co in range(NCH):
                    nc.vector.tensor_scalar_add(
                        xb[:, co, :],
                        x_sb[:, co, 512 * qb : 512 * (qb + 1)],
                        bo4[:, co : co + 1],
                    )
                # den accumulated in f32 (DVE converts fp8 exp tiles on read);
                # stored as f32r so the ones-matmul can consume it directly;
                # kept as two half-sums (one DVE op per exp pair), summed on
                # the PE via two accumulating psd matmuls
                den = pB.tile([128, 2, 512], F32R, tag="den", bufs=2)
                pso = [
                    psB.tile([128, 512], F32, tag="o", bufs=4, name="pso")
                    for _ in range(NCH)
                ]
                # software-pipelined: scores/exp of pair t+1 are emitted
                # before PV of pair t, so the PE has work to chew while the
                # ACT exp latency drains (pss bufs=3 covers the lookahead)
                pt_tiles = []

                def emit_pair(t):
                    pt8 = pB.tile([128, 2, 512], FP8, tag="pt", bufs=3)
                    pt_tiles.append(pt8)
                    for sub in range(2):
                        jt = 2 * t + sub
                        pss = psB.tile([128, 512], F32, tag="s", bufs=2)
                        for ph in range(2):
                            nc.tensor.matmul(
                                pss[:],
                                kt[:, 2 * ph : 2 * ph + 2, 128 * jt : 128 * (jt + 1)],
                                qt[:, 2 * ph : 2 * ph + 2, 512 * qb : 512 * (qb + 1)],
                                start=(ph == 0),
                                stop=(ph == 1),
                                perf_mode=DR,
                            )
                        nc.scalar.activation(
                            pt8[:, sub, :], pss[:], ACTF.Exp,
                            scale=SCALE_Q, bias=expb_t[:],
                        )
                    if t == 0:
                        nc.vector.tensor_copy(den[:], pt8[:])
                    else:
                        nc.vector.tensor_add(
                            den[:], den[:].bitcast(F32), pt8[:]
                        )

                def emit_pv(t):
                    pt8 = pt_tiles[t]
                    for cc in range(NCH):
                        nc.tensor.matmul(
                            pso[cc][:],
                            vt[:, 2 * t : 2 * t + 2, 128 * cc : 128 * (cc + 1)],
                            pt8[:],
                            start=(t == 0),
                            stop=(t == NJT // 2 - 1),
                            perf_mode=DR,
                        )

                emit_pair(0)
                if pending is not None:
                    emit_epilogue(pending)
                    pending = None
                for t in range(1, NJT // 2):
                    emit_pair(t)
                    emit_pv(t - 1)
                emit_pv(NJT // 2 - 1)
                # partition-sum of den, broadcast of 1/den
                psd = psB.tile([1, 512], F32, tag="f", bufs=1)
                nc.tensor.matmul(psd[:], ones128[:], den[:, 0, :], start=True, stop=False)
                nc.tensor.matmul(psd[:], ones128[:], den[:, 1, :], start=False, stop=True)
                dtot = pB.tile([1, 512], F32R, tag="dtot", bufs=2)
                nc.vector.tensor_copy(dtot[:], psd[:])
                # psb gets its own PSUM bank: the reciprocal holds a long read
                # on it and must not gate the next epilogue's psf matmuls
                psb = psB.tile([128, 512], F32, tag="b", bufs=1)
                nc.tensor.matmul(psb[:], ones1[:], dtot[:], start=True, stop=True)
                rbc = pB.tile([128, 512], F32, tag="rbc", bufs=2)
                nc.vector.reciprocal(rbc[:], psb[:])
                # drain unnormalized O^T to SBUF (frees the pso banks);
                # split across ACT and DVE so neither queue stalls the next
                # q-block's exp/den chain
                osb = pB.tile([128, NCH, 512], BF16, tag="osb", bufs=2)
                for cc in range(NCH):
                    if cc % 2 == 0:
                        nc.scalar.activation(osb[:, cc, :], pso[cc][:], ACTF.Identity)
                    else:
                        nc.vector.tensor_copy(osb[:, cc, :], pso[cc][:])
                pending = (qb, osb, rbc, xb)
            if pending is not None:
                emit_epilogue(pending)
                pending = None
    return nc


# ---------------------------------------------------------------------------
# Walrus in this container rejects instructions carrying more than ~2
# sync-wait commands ("Too many sync wait commands").  Hoist excess on_wait
# entries onto nofuse NOPs placed immediately before the instruction on the
# same engine (engines issue in-order, so blocking on the NOP first is
# equivalent).
def split_sync_waits(nc, max_waits=1):
    n_split = 0
    for bb in nc.main_func.blocks:
        insts = bb.instructions
        out = []
        for inst in insts:
            si = inst.sync_info
            if si is not None and si.on_wait is not None and len(si.on_wait) > max_waits:
                waits = list(si.on_wait)
                keep = waits[-max_waits:]
                extra = waits[:-max_waits]
                for i in range(0, len(extra), max_waits):
                    chunk = extra[i : i + max_waits]
                    nop = mybir.InstNoOp(
                        name=f"{inst.name}-sw{i}",
                        sync_info=mybir.SyncInfo(on_wait=chunk, on_update=[]),
                        bass_nofuse=True,
                        engine=inst.engine,
                    )
                    out.append(nop)
                    n_split += 1
                inst.sync_info = mybir.SyncInfo(
                    on_wait=keep, on_update=list(si.on_update or [])
                )
            out.append(inst)
        bb.instructions = out
    return n_split


B, H, W = 8, 64, 64
HW = H * W
N_CORES = 8
_CACHE = {}


def _get_nc():
    if "nc" not in _CACHE:
        nc = bass.Bass()
        build(nc, HW=HW)
        split_sync_waits(nc)
        _CACHE["nc"] = nc
    return _CACHE["nc"]


def _in_maps(inputs):
    import numpy as np
    arrs = {k: np.ascontiguousarray(np.asarray(v, dtype=np.float32)) for k, v in inputs.items()}
    x = arrs.pop("x").reshape(B, C, HW)
    return [{"x": x[i], **arrs} for i in range(N_CORES)]


def kernel(**inputs):
    import numpy as np
    from concourse.bass_utils import run_bass_kernel_spmd

    nc = _get_nc()
    res = run_bass_kernel_spmd(nc, _in_maps(inputs), list(range(N_CORES)))
    out = np.stack([res.results[i]["out"] for i in range(N_CORES)])
    return out.reshape(B, C, H, W).astype(np.float32)


def kernel_traced(**inputs):
    """Like kernel() but with NTFF profiling; returns (output, BassKernelResults)."""
    import numpy as np
    from concourse.bass_utils import run_bass_kernel_spmd

    nc = _get_nc()
    res = run_bass_kernel_spmd(
        nc, _in_maps(inputs), list(range(N_CORES)), trace=True
    )
    out = np.stack([res.results[i]["out"] for i in range(N_CORES)])
    return out.reshape(B, C, H, W).astype(np.float32), res
